# revision 76
# baseline (speedup 1.0000x reference)
"""Trainium2 Bass kernel for nn_DecoderLM_91018946936840.

4-layer pre-LN decoder (D=1024, H=16, S=1024, B=4, ff=4096) on 8 NeuronCores:
data-parallel over B (4 pair-groups) x Megatron-SP tensor-parallel 2 within
each pair: attention is head-sharded (8 heads/core), LayerNorm / residual /
MLP are sequence-sharded (512 tokens/core, full 4096-wide FF, no collective).
Per layer: one AllGather of the LN1 output (bf16) before QKV and one
ReduceScatter (add) after the attention out-projection.

Activations are feature-major [D, tokens]: LayerNorm statistics come from
ones-matmuls, per-token scalars are broadcast across partitions via small
DRAM round-trip DMAs, and each head's softmax denominator rides along the AV
matmul as a ones-column appended to V. Matmuls run bf16 with fp32 PSUM.
"""
import numpy as np
import ml_dtypes

import concourse.bass as bass
import concourse.mybir as mybir
import concourse.tile as tile
from concourse.bass_utils import run_bass_kernel_spmd
from concourse.vector_clock import ScopedClock

# ---------------------------------------------------------------------------
# Workaround: this walrus build accepts at most ONE semaphore wait per
# instruction ("Too many sync wait commands"). Redistribute Tile-assigned
# waits onto single-wait NoOps in front of the owning instruction, and do the
# same for the kernel-tail drain.
# ---------------------------------------------------------------------------
_MAX_WAITS = 1


def _patched_drain_and_barrier(self, tick_clock, wait_clock):
    nc = self.nc
    probe = nc.sync.nop(hint="drain_waits", nofuse=True)
    wait_clock.add_sem_waits(probe.ins, ScopedClock({None: tick_clock.global_clock}))
    si = probe.ins.sync_info
    waits = list(si.on_wait) if si is not None else []
    probe.ins.sync_info = mybir.SyncInfo(
        on_wait=waits[:_MAX_WAITS],
        on_update=list(si.on_update) if si is not None else [],
    )
    for i in range(_MAX_WAITS, len(waits), _MAX_WAITS):
        extra = nc.sync.nop(hint="drain_waits", nofuse=True)
        extra.ins.sync_info = mybir.SyncInfo(
            on_wait=waits[i : i + _MAX_WAITS], on_update=[])
    nc.sync.drain()
    nc.all_engine_barrier()
    assert self.sems is not None
    popped = nc._tile_sem_poison_stack.pop()
    assert popped is self._sem_poison
    nc.clear_and_free_semaphores(list(self.sems.allocated().values()))
    nc.all_engine_barrier()


_orig_commit = tile.TileContext._commit_instruction


def _patched_commit_instruction(self, inst, lazy_reg_writes=True):
    si = inst.sync_info
    if si is not None and len(si.on_wait) > _MAX_WAITS:
        waits = list(si.on_wait)
        keep, extras = waits[-_MAX_WAITS:], waits[:-_MAX_WAITS]
        engine = inst.engine
        if engine == mybir.EngineType.Unassigned:
            engine = mybir.EngineType.SP
        for w in extras:
            nop = mybir.InstNoOp(
                name=self.nc.get_next_instruction_name(),
                ins=[],
                outs=[],
                engine=engine,
                sync_info=mybir.SyncInfo(on_wait=[w], on_update=[]),
            )
            self._add_instruction(nop)
        inst.sync_info = mybir.SyncInfo(on_wait=keep, on_update=list(si.on_update))
    return _orig_commit(self, inst, lazy_reg_writes)


tile.TileContext._drain_and_barrier = _patched_drain_and_barrier
tile.TileContext._commit_instruction = _patched_commit_instruction

# ---------------------------------------------------------------------------

V, D, H, L, B, S = 32000, 1024, 16, 4, 4, 1024
HD = D // H          # 64
FF = 4 * D           # 4096
EPS = 1e-5
N_CORES = 8
SH = D // 2          # 512   qkv output shard per core (8 heads)
NT = D // 128        # 8     model-dim tiles
NQ = SH // 128       # 4     shard-dim tiles
NM = FF // 128       # 32    full-ffn m-tiles
CH = 512             # token chunk == own token half
NCH = S // CH        # 2
HL = 8               # heads per core

F32 = mybir.dt.float32
F32R = mybir.dt.float32r
BF16 = mybir.dt.bfloat16
ADD = mybir.AluOpType.add
MULT = mybir.AluOpType.mult
SUB = mybir.AluOpType.subtract
BYPASS = mybir.AluOpType.bypass
F8 = mybir.dt.float8e4
AF = mybir.ActivationFunctionType

REPLICA_GROUPS = [[0, 1], [2, 3], [4, 5], [6, 7]]


def build_nc():
    nc = bass.Bass(trn_type="TRN2", target_bir_lowering=False, debug=False,
                   num_devices=N_CORES)

    def inp(name, shape, dt=F32):
        return nc.dram_tensor(name, list(shape), dt, kind="ExternalInput")

    x0t = inp("x0t", [D, CH])
    x0f_d = inp("x0f", [NCH, 128, NT, CH], BF16)
    wq_d = inp("wq", [L, D, SH], BF16)
    wk_d = inp("wk", [L, D, SH], BF16)
    wv_d = inp("wv", [L, D, SH], BF16)
    wo_d = inp("wo", [L, SH, D], BF16)
    w1_d = inp("w1", [L, D, FF], BF16)
    w2_d = inp("w2", [L, FF, D], BF16)
    bq_d = inp("bq", [L, 128, NQ])
    bk_d = inp("bk", [L, 128, NQ])
    bv_d = inp("bv", [L, 128, NQ])
    bo_d = inp("bo2", [L, 128, NT])     # pre-halved (RS sums over the pair)
    b1_d = inp("b1", [L, 128, NM])
    b2_d = inp("b2f", [L, 128, NT])     # full (no collective after w2)
    g1_d = inp("g1", [L, 128, NT])
    be1_d = inp("be1", [L, 128, NT])
    g2_d = inp("g2", [L, 128, NT])
    be2_d = inp("be2", [L, 128, NT])
    gf_d = inp("gf", [128, NT])
    bef_d = inp("bef", [128, NT])
    mask_d = inp("masks", [128, 4, CH])
    ones_d = inp("cones", [128, 128])

    out_ext = nc.dram_tensor("outt", [D, CH], F32, kind="ExternalOutput")
    out_v = out_ext.ap().rearrange("(t p) s -> p t s", p=128)

    with tile.TileContext(nc) as tc:
        with (
            nc.allow_low_precision(reason="bf16 matmuls + bf16 collectives"),
            tc.tile_pool(name="singles", bufs=1) as singles,
            tc.tile_pool(name="acts", bufs=1) as acts,
            tc.tile_pool(name="big", bufs=1) as big,
            tc.tile_pool(name="qkvw", bufs=1) as qkvw,
            tc.tile_pool(name="w1s", bufs=2) as w1s,
            tc.tile_pool(name="w2s", bufs=2) as w2s,
            tc.tile_pool(name="wt", bufs=8) as wtp,
            tc.tile_pool(name="rows", bufs=2) as rows,
            tc.tile_pool(name="evac", bufs=3) as evac,
            tc.tile_pool(name="pp", bufs=3, space="PSUM") as pp,
            tc.tile_pool(name="pav", bufs=2, space="PSUM") as pav,
            tc.tile_pool(name="prow", bufs=1, space="PSUM") as prow,
            tc.tile_pool(name="dram", bufs=2, space="DRAM") as dram,
        ):
            # ---- resident constants / state -----------------------------
            xT = singles.tile([128, NT, CH], F32R)
            nc.sync.dma_start(
                out=xT[:],
                in_=x0t.ap().rearrange("(t p) s -> p t s", p=128).bitcast(F32R))
            masks = singles.tile([128, 4, CH], BF16)
            nc.gpsimd.dma_start(out=masks[:], in_=mask_d.ap())
            onesr = singles.tile([128, 2], F32R)
            nc.sync.dma_start(out=onesr[:], in_=ones_d.ap()[:, 0:2].bitcast(F32R))
            onesb = singles.tile([1, 128], F32R)
            nc.sync.dma_start(out=onesb[:], in_=ones_d.ap()[0:1, :].bitcast(F32R))
            ones_kb_t = singles.tile([128, 1], BF16)
            nc.vector.memset(ones_kb_t[:], 1.0)
            ones_bcb_t = singles.tile([1, 128], BF16)
            nc.vector.memset(ones_bcb_t[:], 1.0)
            eps_t = singles.tile([1, 1], F32)
            nc.vector.memset(eps_t[:], EPS)
            invD_t = singles.tile([1, 1], F32)
            nc.vector.memset(invD_t[:], 1.0 / D)

            # V stripes [128, NT, 8*65]: per head 64 value cols + ones col
            # (softmax denominator rides the AV matmul). Ones written once.
            Vt = singles.tile([128, NT, HL * 65], BF16)
            nc.gpsimd.dma_start(
                out=Vt[:].rearrange("p t (h c) -> p t h c", h=HL)[:, :, :, 64:65],
                in_=ones_d.ap()[:, 0 : NT * HL]
                    .rearrange("p (t h o) -> p t h o", t=NT, h=HL))

            def load_pp(d, shape):
                t = singles.tile(list(shape), F32, name=f"pp_{d.name}")
                src = d.ap()
                if len(shape) == 3:
                    src = src.rearrange("l p m -> p l m")
                nc.sync.dma_start(out=t[:], in_=src)
                return t

            bqT = load_pp(bq_d, [128, L, NQ])
            bkT = load_pp(bk_d, [128, L, NQ])
            bvT = load_pp(bv_d, [128, L, NQ])
            boT = load_pp(bo_d, [128, L, NT])
            b1T = load_pp(b1_d, [128, L, NM])
            b2T = load_pp(b2_d, [128, L, NT])
            g1T = load_pp(g1_d, [128, L, NT])
            be1T = load_pp(be1_d, [128, L, NT])
            g2T = load_pp(g2_d, [128, L, NT])
            be2T = load_pp(be2_d, [128, L, NT])
            gfT = load_pp(gf_d, [128, NT])
            befT = load_pp(bef_d, [128, NT])

            ones_k = onesr[:, 0:1]            # [128,1] lhsT for column sums
            ones_kb = ones_kb_t[:, 0:1]       # [128,1] bf16 lhsT for sums
            ones_bc = onesb[:, :]             # [1,128] lhsT for broadcasts
            ones_bcb = ones_bcb_t[:, :]       # [1,128] bf16 lhsT broadcasts

            # ---- layernorm over the feature dim (512 tokens) ------------
            def ln_rowstats(psx, psx2, out01):
                """psx/psx2 PSUM sum rows -> out01 [2,CH] = [1/sigma, m/sigma].

                Chain kept on DVE (one Act hop for sqrt) to minimize
                cross-engine semaphore latency.
                """
                mrow = rows.tile([1, CH], F32R, tag="mr")
                nc.vector.tensor_scalar_mul(out=mrow[:], in0=psx[:],
                                            scalar1=invD_t[:])
                m2row = rows.tile([1, CH], F32R, tag="rb")
                nc.vector.tensor_tensor(out=m2row[:], in0=mrow[:],
                                        in1=mrow[:], op=MULT)
                vrow = rows.tile([1, CH], F32R, tag="rb")
                nc.vector.tensor_scalar_mul(out=vrow[:], in0=psx2[:],
                                            scalar1=invD_t[:])
                nc.vector.tensor_tensor(out=vrow[:], in0=vrow[:],
                                        in1=m2row[:], op=SUB)
                srow = rows.tile([1, CH], F32R, tag="rb")
                nc.scalar.activation(out=srow[:], in_=vrow[:],
                                     func=AF.Sqrt, bias=eps_t[:], scale=1.0)
                rr, mrr = out01
                nc.vector.reciprocal(out=rr[:], in_=srow[:])
                nc.vector.tensor_tensor(out=mrr[:], in0=mrow[:],
                                        in1=rr[:], op=MULT)

            def ln_norm(st01, gT, bT, l_idx, dest_fn, src, bdt=BF16):
                """normalize src tiles with stats rows (rr, mrr) -> dest_fn."""
                rr, mrr = st01
                blhs = ones_bcb if rr.dtype == BF16 else ones_bc
                prbP = prow.tile([128, CH], F32, tag="bc", bufs=1)
                nc.tensor.matmul(prbP[:], blhs, rr[:],
                                 start=True, stop=True)
                prb = evac.tile([128, CH], bdt, tag="rbb", bufs=1)
                nc.scalar.activation(out=prb[:], in_=prbP[:],
                                     func=AF.Identity, scale=1.0)
                pmrbP = prow.tile([128, CH], F32, tag="bc", bufs=1)
                nc.tensor.matmul(pmrbP[:], blhs, mrr[:],
                                 start=True, stop=True)
                pmrb = evac.tile([128, CH], bdt, tag="mrbb", bufs=1)
                nc.scalar.activation(out=pmrb[:], in_=pmrbP[:],
                                     func=AF.Identity, scale=1.0)
                for t in range(NT):
                    tmp = evac.tile([128, CH], bdt, tag="lntmp", bufs=3)
                    nc.vector.tensor_tensor(out=tmp[:], in0=src[:, t, :],
                                            in1=prb[:], op=MULT)
                    nc.vector.tensor_tensor(out=tmp[:], in0=tmp[:],
                                            in1=pmrb[:], op=SUB)
                    if l_idx is not None:
                        gs = gT[:, l_idx, t : t + 1]
                        bs = bT[:, l_idx, t : t + 1]
                    else:
                        gs = gT[:, t : t + 1]
                        bs = bT[:, t : t + 1]
                    dest_fn(t, tmp, gs, bs)

            def layernorm(gT, bT, l_idx, dest_fn, src=None, bdt=BF16):
                if src is None:
                    src = xT
                sum_lhs = ones_kb if src.dtype == BF16 else ones_k
                psx = prow.tile([1, CH], F32, tag="psx")
                psx2 = prow.tile([1, CH], F32, tag="psx2")
                for t in range(NT):
                    nc.tensor.matmul(psx[:], sum_lhs, src[:, t, :],
                                     start=(t == 0), stop=(t == NT - 1))
                for t in range(NT):
                    sq = evac.tile([128, CH], BF16, tag="lntmp", bufs=3)
                    nc.scalar.activation(out=sq[:], in_=src[:, t, :],
                                         func=AF.Square, scale=1.0)
                    nc.tensor.matmul(psx2[:], ones_kb, sq[:],
                                     start=(t == 0), stop=(t == NT - 1))
                rr = rows.tile([1, CH], F32R, tag="rrow")
                mrr = rows.tile([1, CH], F32R, tag="mrow2", bufs=1)
                ln_rowstats(psx, psx2, (rr, mrr))
                ln_norm((rr, mrr), gT, bT, l_idx, dest_fn, src, bdt=bdt)

            def load_w(dram_t, l, shape, rearr="(t p) m -> p t m", tag=None,
                       pool=qkvw, bufs=None, cols=None):
                t = pool.tile(list(shape), BF16, tag=tag or dram_t.name,
                              **({"bufs": bufs} if bufs else {}))
                src = dram_t.ap()[l]
                if cols is not None:
                    src = src[:, cols]
                nc.sync.dma_start(out=t[:], in_=src.rearrange(rearr, p=128))
                return t

            # Cross-pair x exchange for the next layer's LN1+QKV, in two
            # pieces: (1) post-attention x (bf16), gathered during the MLP
            # (fully hidden); (2) only the MLP delta (fp8), gathered in two
            # feature halves right as the w2 m-tiles finish, so just ~28us
            # of the second half is exposed. The fp8 half also carries the
            # next layer's LN1 row stats (computed from own f32 x) as a
            # bf16-bitcast tail, so the receiver skips the stats pass.
            XQT = 128 * CH                     # elems per feature tile
            FTA = 4 * XQT                      # fp8 delta elems, first AG
            FTB = 4 * XQT                      # fp8 delta elems, second AG
            FST = 4 * CH                       # stats tail (2xCH bf16 bytes)

            def ffd_copy(gin, t, po):
                pof = evac.tile([128, CH], F8, tag="po", bufs=2)
                nc.vector.tensor_copy(out=pof[:], in_=po[:])
                o = (t % 4) * XQT
                nc.sync.dma_start(
                    out=gin[o:o + XQT].rearrange("(p q) -> p q", p=128),
                    in_=pof[:])

            def gather_go(gin, lnext, half):
                n = (FTB + FST) if half == 1 else FTA
                gout = dram.tile([2, n], F8, tag=f"agxo{half}",
                                 name=f"agxo{lnext}_{half}")
                nc.gpsimd.collective_compute(
                    "AllGather", BYPASS, replica_groups=REPLICA_GROUPS,
                    ins=[gin.opt()], outs=[gout.opt()])
                return gout

            # ---- main body ----------------------------------------------
            agx_prev = None
            for l in range(L):
                # LN1 over both token chunks from the gathered (bf16) x;
                # identical h on both pair members, no h collective.
                hT = big.tile([128, NT, S], BF16, tag="big", name=f"hT{l}")
                for c in range(NCH):
                    xp = acts.tile([128, NT, CH], BF16, tag="h2", bufs=2,
                                   name=f"xp{l}_{c}")

                    def wr_hc(t, tmp, gs, bs, c=c, hT=hT):
                        nc.vector.tensor_scalar(
                            out=hT[:, t, c * CH:(c + 1) * CH], in0=tmp[:],
                            scalar1=gs, scalar2=bs, op0=MULT, op1=ADD)

                    if l == 0:
                        nc.sync.dma_start(out=xp[:], in_=x0f_d.ap()[c])
                        layernorm(g1T, be1T, l, wr_hc, src=xp)
                    else:
                        gxp, ga, gb = agx_prev
                        nc.gpsimd.dma_start(
                            out=xp[:],
                            in_=gxp[c].rearrange("(t p q) -> p t q",
                                                 t=NT, p=128))
                        st_r = rows.tile([1, CH], BF16, tag="strowr",
                                         bufs=1, name=f"strowr{l}_{c}")
                        st_m = rows.tile([1, CH], BF16, tag="strowm",
                                         bufs=1, name=f"strowm{l}_{c}")
                        nc.gpsimd.dma_start(
                            out=st_r[:],
                            in_=gb[c, FTB:FTB + 2 * CH].bitcast(BF16)
                                .rearrange("(r q) -> r q", r=1))
                        nc.gpsimd.dma_start(
                            out=st_m[:],
                            in_=gb[c, FTB + 2 * CH:FTB + FST].bitcast(BF16)
                                .rearrange("(r q) -> r q", r=1))
                        for t in range(NT):
                            g = ga if t < 4 else gb
                            o = (t % 4) * XQT
                            fft = evac.tile([128, CH], F8, tag="po", bufs=2)
                            nc.gpsimd.dma_start(
                                out=fft[:],
                                in_=g[c, o:o + XQT].rearrange("(p q) -> p q",
                                                              p=128))
                            nc.vector.tensor_tensor(
                                out=xp[:, t, :], in0=xp[:, t, :],
                                in1=fft[:], op=ADD)
                        ln_norm((st_r, st_m), g1T, be1T, l, wr_hc, src=xp)

                # per-layer weight tiles (single DMAs, 1KB contiguous rows)
                wqL = load_w(wq_d, l, [128, NT, SH])
                wkL = load_w(wk_d, l, [128, NT, SH])
                wvL = load_w(wv_d, l, [128, NT, SH])
                woL = load_w(wo_d, l, [128, NQ, D])

                # K projection -> feature-major [128, NQ, S] bf16
                # (chunk-major so chunk 0 attention can start early)
                KT = acts.tile([128, NQ, S], BF16, tag="kt")
                for c in range(NCH):
                    for m in range(NQ):
                        cs = slice(c * CH, (c + 1) * CH)
                        ps = pp.tile([128, CH], F32, tag="pp")
                        for k in range(NT):
                            nc.tensor.matmul(
                                ps[:], wkL[:, k, m * 128:(m + 1) * 128],
                                hT[:, k, cs],
                                start=(k == 0), stop=(k == NT - 1))
                        nc.scalar.activation(
                            out=KT[:, m, cs], in_=ps[:], func=AF.Identity,
                            bias=bkT[:, l, m : m + 1], scale=1.0)

                # V projection -> token-major stripes (64 cols + ones col)
                def v_proj(mt):
                    ps = pp.tile([128, SH], F32, tag="pp")
                    for k in range(NT):
                        nc.tensor.matmul(
                            ps[:], hT[:, k, mt * 128:(mt + 1) * 128],
                            wvL[:, k, :],
                            start=(k == 0), stop=(k == NT - 1))
                    nc.vector.tensor_copy(
                        out=Vt[:, mt, :].rearrange("p (h c) -> p h c", h=HL)[:, :, 0:64],
                        in_=ps[:].rearrange("p (h c) -> p h c", h=HL))

                for mt in range(4):
                    v_proj(mt)

                # attention -> attnT [128, NQ, S] bf16, then wo partials
                attnT = acts.tile([128, NQ, S], BF16, tag="at")
                rs_in = dram.tile([NCH, 128, NT, CH], BF16, tag="rsin")
                rs_out = dram.tile([128, NT, CH], BF16, tag="rsout")
                for c in range(NCH):
                    cs = slice(c * CH, (c + 1) * CH)
                    nk = 4 * c + 4
                    QTc = acts.tile([128, NQ, CH], BF16, tag="qt", bufs=1)
                    for m in range(NQ):
                        ps = pp.tile([128, CH], F32, tag="pp")
                        for k in range(NT):
                            nc.tensor.matmul(
                                ps[:], wqL[:, k, m * 128:(m + 1) * 128],
                                hT[:, k, cs],
                                start=(k == 0), stop=(k == NT - 1))
                        nc.scalar.activation(
                            out=QTc[:, m, :], in_=ps[:], func=AF.Identity,
                            bias=bqT[:, l, m : m + 1], scale=1.0)
                    if c == 0:
                        for mt in range(4, NT):
                            v_proj(mt)
                    for h in range(HL):
                        base = 64 * (h % 2)
                        hp = h // 2
                        wts = []
                        for j in range(nk):
                            pl = pp.tile([128, CH], F32, tag="pp")
                            nc.tensor.matmul(
                                pl[:],
                                KT[base:base + 64, hp, j * 128:(j + 1) * 128],
                                QTc[base:base + 64, hp, :],
                                start=True, stop=True)
                            wt = wtp.tile([128, CH], BF16, tag="wt", bufs=8)
                            nc.scalar.activation(out=wt[:], in_=pl[:],
                                                 func=AF.Exp, scale=0.125)
                            r = j - 4 * c
                            if r >= 0:
                                nc.vector.tensor_tensor(
                                    out=wt[:], in0=wt[:],
                                    in1=masks[:, r, :], op=MULT)
                            wts.append(wt)
                        pa = pav.tile([65, CH], F32, tag="pav")
                        for j in range(nk):
                            nc.tensor.matmul(
                                pa[:], Vt[:, j, 65 * h : 65 * h + 65],
                                wts[j][:],
                                start=(j == 0), stop=(j == nk - 1))
                        rec = rows.tile([1, CH], F32R, tag="rb")
                        nc.vector.reciprocal(out=rec[:], in_=pa[64:65, :])
                        # broadcast 1/denominator across partitions via a
                        # K=1 ones-matmul into the shared bc PSUM bank, then
                        # to SBUF (DVE cannot read two PSUM operands)
                        pbc = prow.tile([64, CH], F32, tag="bc", bufs=1)
                        nc.tensor.matmul(pbc[:], ones_bc[:, 0:64],
                                         rec[:], start=True, stop=True)
                        recb = wtp.tile([64, CH], BF16, tag="recb", bufs=2)
                        nc.vector.tensor_copy(out=recb[:], in_=pbc[:])
                        nc.vector.tensor_tensor(
                            out=attnT[base:base + 64, hp, cs],
                            in0=pa[0:64, :], in1=recb[:], op=MULT)
                    for t in range(NQ):
                        nc.vector.tensor_scalar_add(
                            out=attnT[:, t, cs], in0=attnT[:, t, cs],
                            scalar1=bvT[:, l, t : t + 1])
                    # wo partials for this chunk -> ReduceScatter segment c
                    for m in range(NT):
                        ps = pp.tile([128, CH], F32, tag="pp")
                        for k in range(NQ):
                            nc.tensor.matmul(
                                ps[:], woL[:, k, m * 128:(m + 1) * 128],
                                attnT[:, k, cs],
                                start=(k == 0), stop=(k == NQ - 1))
                        po = evac.tile([128, CH], BF16, tag="po", bufs=2)
                        if c == 0 or m >= 6:
                            nc.scalar.activation(
                                out=po[:], in_=ps[:], func=AF.Identity,
                                bias=boT[:, l, m : m + 1], scale=1.0)
                        else:
                            nc.vector.tensor_scalar_add(
                                out=po[:], in0=ps[:],
                                scalar1=boT[:, l, m : m + 1])
                        nc.sync.dma_start(out=rs_in[c, :, m, :], in_=po[:])
                nc.gpsimd.collective_compute(
                    "ReduceScatter", ADD, replica_groups=REPLICA_GROUPS,
                    ins=[rs_in.opt()], outs=[rs_out.opt()])

                # residual(x) += RS result, then LN2 -> h2 (bf16).
                # Post-attention x is copied out per tile and AllGathered
                # during the MLP (fully hidden).
                rsS = acts.tile([128, NT, CH], BF16, tag="h2", bufs=2,
                                name=f"rsS{l}")
                nc.sync.dma_start(out=rsS[:, 0:4, :], in_=rs_out[:, 0:4, :])
                nc.sync.dma_start(out=rsS[:, 4:8, :], in_=rs_out[:, 4:8, :])
                for t in range(NT):
                    nc.vector.tensor_tensor(
                        out=xT[:, t, :], in0=xT[:, t, :],
                        in1=rsS[:, t, :], op=ADD)
                h2 = acts.tile([128, NT, CH], BF16, tag="h2", bufs=2)

                def wr_h2(t, tmp, gs, bs, h2=h2):
                    nc.vector.tensor_scalar(
                        out=h2[:, t, :], in0=tmp[:],
                        scalar1=gs, scalar2=bs, op0=MULT, op1=ADD)

                layernorm(g2T, be2T, l, wr_h2)

                # post-attention x copy-out + hidden AllGather (during MLP)
                if l + 1 < L:
                    gxp_in = dram.tile([NT * XQT], BF16, tag="gxpi",
                                       name=f"gxpi{l + 1}")
                    for t in range(NT):
                        xc = evac.tile([128, CH], BF16, tag="po", bufs=2)
                        nc.scalar.activation(out=xc[:], in_=xT[:, t, :],
                                             func=AF.Identity, scale=1.0)
                        nc.sync.dma_start(
                            out=gxp_in[t * XQT:(t + 1) * XQT]
                                .rearrange("(p q) -> p q", p=128),
                            in_=xc[:])
                    gxp_out = dram.tile([2, NT * XQT], BF16, tag="gxpo",
                                        name=f"gxpo{l + 1}")
                    nc.gpsimd.collective_compute(
                        "AllGather", BYPASS, replica_groups=REPLICA_GROUPS,
                        ins=[gxp_in.opt()], outs=[gxp_out.opt()])

                # MLP: full 4096-wide FF over own 512 tokens, no collective
                gT = big.tile([128, NM, CH], BF16, tag="big", name=f"gT{l}")
                for mm in range(16):
                    w1t = w1s.tile([128, NT, 256], BF16, tag="w1")
                    nc.sync.dma_start(
                        out=w1t[:],
                        in_=w1_d.ap()[l][:, mm * 256:(mm + 1) * 256]
                            .rearrange("(t p) m -> p t m", p=128))
                    for ms in range(2):
                        m = mm * 2 + ms
                        ps = pp.tile([128, CH], F32, tag="pp")
                        for k in range(NT):
                            nc.tensor.matmul(
                                ps[:], w1t[:, k, ms * 128:(ms + 1) * 128],
                                h2[:, k, :],
                                start=(k == 0), stop=(k == NT - 1))
                        nc.scalar.activation(
                            out=gT[:, m, :], in_=ps[:], func=AF.Gelu,
                            bias=b1T[:, l, m : m + 1], scale=1.0)
                agx_a = None
                if l + 1 < L:
                    gin_a = dram.tile([FTA], F8, tag="agxi0",
                                      name=f"agxi{l + 1}_0")
                    gin_b = dram.tile([FTB + FST], F8, tag="agxi1",
                                      name=f"agxi{l + 1}_1")
                    psxN = prow.tile([1, CH], F32, tag="psx")
                    psx2N = prow.tile([1, CH], F32, tag="psx2")
                for m2 in range(4):
                    w2t = w2s.tile([128, NM, 256], BF16, tag="w2")
                    nc.sync.dma_start(
                        out=w2t[:],
                        in_=w2_d.ap()[l][:, m2 * 256:(m2 + 1) * 256]
                            .rearrange("(t p) m -> p t m", p=128))
                    for ms in range(2):
                        m = m2 * 2 + ms
                        ps = pp.tile([128, CH], F32, tag="pp")
                        for k in range(NM):
                            nc.tensor.matmul(
                                ps[:], w2t[:, k, ms * 128:(ms + 1) * 128],
                                gT[:, k, :],
                                start=(k == 0), stop=(k == NM - 1))
                        po = evac.tile([128, CH], BF16, tag="po", bufs=2)
                        nc.scalar.activation(
                            out=po[:], in_=ps[:], func=AF.Identity,
                            bias=b2T[:, l, m : m + 1], scale=1.0)
                        nc.vector.tensor_tensor(
                            out=xT[:, m, :], in0=xT[:, m, :],
                            in1=po[:], op=ADD)
                        if l + 1 < L:
                            ffd_copy(gin_a if m < 4 else gin_b, m, po)
                            nc.tensor.matmul(psxN[:], ones_k, xT[:, m, :],
                                             start=(m == 0),
                                             stop=(m == NT - 1))
                            sq = evac.tile([128, CH], BF16, tag="lntmp",
                                           bufs=3)
                            nc.vector.tensor_tensor(out=sq[:],
                                                    in0=xT[:, m, :],
                                                    in1=xT[:, m, :],
                                                    op=MULT)
                            nc.tensor.matmul(psx2N[:], ones_kb, sq[:],
                                             start=(m == 0),
                                             stop=(m == NT - 1))
                            if m == 3:
                                agx_a = gather_go(gin_a, l + 1, 0)
                if l + 1 < L:
                    stb_r = rows.tile([1, CH], BF16, tag="stbr", bufs=1,
                                      name=f"stbr{l + 1}")
                    stb_m = rows.tile([1, CH], BF16, tag="stbm", bufs=1,
                                      name=f"stbm{l + 1}")
                    ln_rowstats(psxN, psx2N, (stb_r, stb_m))
                    nc.sync.dma_start(
                        out=gin_b[FTB:FTB + 2 * CH].bitcast(BF16)
                            .rearrange("(r q) -> r q", r=1),
                        in_=stb_r[:])
                    nc.sync.dma_start(
                        out=gin_b[FTB + 2 * CH:FTB + FST].bitcast(BF16)
                            .rearrange("(r q) -> r q", r=1),
                        in_=stb_m[:])
                    agx_prev = (gxp_out, agx_a,
                                gather_go(gin_b, l + 1, 1))

            # final LN -> output (own token half)
            def wr_out(t, tmp, gs, bs):
                ot = evac.tile([128, CH], F32, tag="ot", bufs=1)
                nc.vector.tensor_scalar(out=ot[:], in0=tmp[:],
                                        scalar1=gs, scalar2=bs,
                                        op0=MULT, op1=ADD)
                nc.sync.dma_start(out=out_v[:, t, :], in_=ot[:])

            layernorm(gfT, befT, None, wr_out)

    return nc


# ---------------------------------------------------------------------------
# host side
# ---------------------------------------------------------------------------

def _sinusoidal_pe(s, d):
    pos = np.arange(s, dtype=np.float32)[:, None]
    div = np.exp(np.arange(0, d, 2, dtype=np.float32)
                 * np.float32(-np.log(10000.0) / d)).astype(np.float32)
    pe = np.zeros((s, d), dtype=np.float32)
    pe[:, 0::2] = np.sin(pos * div)
    pe[:, 1::2] = np.cos(pos * div)
    return pe


def _pp128(v):
    """[L?, n*128] -> [L?, 128, n] with feature = 128*m + p."""
    v = np.asarray(v, dtype=np.float32)
    if v.ndim == 1:
        return np.ascontiguousarray(v.reshape(-1, 128).T)
    lq, n = v.shape
    return np.ascontiguousarray(v.reshape(lq, n // 128, 128).transpose(0, 2, 1))


_NC_CACHE = {}


def _get_nc(repeat=1):
    if repeat not in _NC_CACHE:
        _NC_CACHE[repeat] = build_nc()
    return _NC_CACHE[repeat]


def make_in_maps(input_ids, tok_emb, wq, bq, wk, bk, wv, bv, wo, bo,
                 ln1_g, ln1_b, ln2_g, ln2_b, w1, b1, w2, b2, lnf_g, lnf_b):
    input_ids = np.asarray(input_ids)
    pe = _sinusoidal_pe(S, D)
    masks = np.zeros((128, 4, CH), dtype=np.float32)
    ar = np.arange(CH)
    for r in range(4):
        for p in range(128):
            masks[p, r, :] = (ar >= 128 * r + p).astype(np.float32)
    cones = np.ones((128, 128), dtype=np.float32)
    bf = ml_dtypes.bfloat16

    in_maps = []
    for core in range(N_CORES):
        b = core // 2
        j = core % 2
        qs = slice(j * SH, (j + 1) * SH)
        ts = slice(j * CH, (j + 1) * CH)
        x0 = (tok_emb[input_ids[b]] + pe).astype(np.float32)   # [S, D]
        x0f = np.ascontiguousarray(
            x0.T.reshape(NT, 128, NCH, CH).transpose(2, 1, 0, 3)).astype(bf)
        m = {
            "x0t": np.ascontiguousarray(x0.T[:, ts]),
            "x0f": x0f,
            "wq": np.ascontiguousarray(wq[:, :, qs]).astype(bf),
            "wk": np.ascontiguousarray(wk[:, :, qs]).astype(bf),
            "wv": np.ascontiguousarray(wv[:, :, qs]).astype(bf),
            "wo": np.ascontiguousarray(wo[:, qs, :]).astype(bf),
            "w1": np.ascontiguousarray(w1).astype(bf),
            "w2": np.ascontiguousarray(w2).astype(bf),
            "bq": _pp128(bq[:, qs]),
            "bk": _pp128(bk[:, qs]),
            "bv": _pp128(bv[:, qs]),
            "bo2": _pp128(bo * 0.5),
            "b1": _pp128(b1),
            "b2f": _pp128(b2),
            "g1": _pp128(ln1_g),
            "be1": _pp128(ln1_b),
            "g2": _pp128(ln2_g),
            "be2": _pp128(ln2_b),
            "gf": _pp128(lnf_g),
            "bef": _pp128(lnf_b),
            "masks": masks,
            "cones": cones,
        }
        in_maps.append(m)
    return in_maps


def kernel(input_ids, attention_mask, tok_emb, ln1_g, ln1_b, wq, bq, wk, bk,
           wv, bv, wo, bo, ln2_g, ln2_b, w1, b1, w2, b2, lnf_g, lnf_b,
           _repeat=1):
    args = [np.asarray(a, dtype=np.float32) for a in
            (tok_emb, wq, bq, wk, bk, wv, bv, wo, bo,
             ln1_g, ln1_b, ln2_g, ln2_b, w1, b1, w2, b2, lnf_g, lnf_b)]
    (tok_emb, wq, bq, wk, bk, wv, bv, wo, bo,
     ln1_g, ln1_b, ln2_g, ln2_b, w1, b1, w2, b2, lnf_g, lnf_b) = args
    in_maps = make_in_maps(input_ids, tok_emb, wq, bq, wk, bk, wv, bv, wo, bo,
                           ln1_g, ln1_b, ln2_g, ln2_b, w1, b1, w2, b2,
                           lnf_g, lnf_b)
    nc = _get_nc(_repeat)
    res = run_bass_kernel_spmd(nc, in_maps, list(range(N_CORES)))
    out = np.empty((B, S, D), dtype=np.float32)
    for b in range(B):
        for j in range(2):
            out[b, j * CH:(j + 1) * CH] = res.results[2 * b + j]["outt"].T
    return out


# revision 83
# speedup vs baseline: 1.0119x; 1.0119x over previous
"""Trainium2 Bass kernel for nn_DecoderLM_91018946936840.

4-layer pre-LN decoder (D=1024, H=16, S=1024, B=4, ff=4096) on 8 NeuronCores:
data-parallel over B (4 pair-groups) x Megatron-SP tensor-parallel 2 within
each pair: attention is head-sharded (8 heads/core), LayerNorm / residual /
MLP are sequence-sharded (512 tokens/core, full 4096-wide FF, no collective).
Per layer: one AllGather of the LN1 output (bf16) before QKV and one
ReduceScatter (add) after the attention out-projection.

Activations are feature-major [D, tokens]: LayerNorm statistics come from
ones-matmuls, per-token scalars are broadcast across partitions via small
DRAM round-trip DMAs, and each head's softmax denominator rides along the AV
matmul as a ones-column appended to V. Matmuls run bf16 with fp32 PSUM.
"""
import numpy as np
import ml_dtypes

import concourse.bass as bass
import concourse.mybir as mybir
import concourse.tile as tile
from concourse.bass_utils import run_bass_kernel_spmd
from concourse.vector_clock import ScopedClock

# ---------------------------------------------------------------------------
# Workaround: this walrus build accepts at most ONE semaphore wait per
# instruction ("Too many sync wait commands"). Redistribute Tile-assigned
# waits onto single-wait NoOps in front of the owning instruction, and do the
# same for the kernel-tail drain.
# ---------------------------------------------------------------------------
_MAX_WAITS = 1


def _patched_drain_and_barrier(self, tick_clock, wait_clock):
    nc = self.nc
    probe = nc.sync.nop(hint="drain_waits", nofuse=True)
    wait_clock.add_sem_waits(probe.ins, ScopedClock({None: tick_clock.global_clock}))
    si = probe.ins.sync_info
    waits = list(si.on_wait) if si is not None else []
    probe.ins.sync_info = mybir.SyncInfo(
        on_wait=waits[:_MAX_WAITS],
        on_update=list(si.on_update) if si is not None else [],
    )
    for i in range(_MAX_WAITS, len(waits), _MAX_WAITS):
        extra = nc.sync.nop(hint="drain_waits", nofuse=True)
        extra.ins.sync_info = mybir.SyncInfo(
            on_wait=waits[i : i + _MAX_WAITS], on_update=[])
    nc.sync.drain()
    nc.all_engine_barrier()
    assert self.sems is not None
    popped = nc._tile_sem_poison_stack.pop()
    assert popped is self._sem_poison
    nc.clear_and_free_semaphores(list(self.sems.allocated().values()))
    nc.all_engine_barrier()


_orig_commit = tile.TileContext._commit_instruction


def _patched_commit_instruction(self, inst, lazy_reg_writes=True):
    si = inst.sync_info
    if si is not None and len(si.on_wait) > _MAX_WAITS:
        waits = list(si.on_wait)
        keep, extras = waits[-_MAX_WAITS:], waits[:-_MAX_WAITS]
        engine = inst.engine
        if engine == mybir.EngineType.Unassigned:
            engine = mybir.EngineType.SP
        for w in extras:
            nop = mybir.InstNoOp(
                name=self.nc.get_next_instruction_name(),
                ins=[],
                outs=[],
                engine=engine,
                sync_info=mybir.SyncInfo(on_wait=[w], on_update=[]),
            )
            self._add_instruction(nop)
        inst.sync_info = mybir.SyncInfo(on_wait=keep, on_update=list(si.on_update))
    return _orig_commit(self, inst, lazy_reg_writes)


tile.TileContext._drain_and_barrier = _patched_drain_and_barrier
tile.TileContext._commit_instruction = _patched_commit_instruction

# ---------------------------------------------------------------------------

V, D, H, L, B, S = 32000, 1024, 16, 4, 4, 1024
HD = D // H          # 64
FF = 4 * D           # 4096
EPS = 1e-5
N_CORES = 8
SH = D // 2          # 512   qkv output shard per core (8 heads)
NT = D // 128        # 8     model-dim tiles
NQ = SH // 128       # 4     shard-dim tiles
NM = FF // 128       # 32    full-ffn m-tiles
CH = 512             # token chunk == own token half
NCH = S // CH        # 2
HL = 8               # heads per core

F32 = mybir.dt.float32
F32R = mybir.dt.float32r
BF16 = mybir.dt.bfloat16
ADD = mybir.AluOpType.add
MULT = mybir.AluOpType.mult
SUB = mybir.AluOpType.subtract
BYPASS = mybir.AluOpType.bypass
F8 = mybir.dt.float8e4
AF = mybir.ActivationFunctionType

REPLICA_GROUPS = [[0, 1], [2, 3], [4, 5], [6, 7]]


def build_nc():
    nc = bass.Bass(trn_type="TRN2", target_bir_lowering=False, debug=False,
                   num_devices=N_CORES)

    def inp(name, shape, dt=F32):
        return nc.dram_tensor(name, list(shape), dt, kind="ExternalInput")

    x0t = inp("x0t", [D, CH])
    x0f_d = inp("x0f", [NCH, 128, NT, CH], BF16)
    st0_d = inp("st0", [NCH, 2, CH], BF16)
    wq_d = inp("wq", [L, D, SH], BF16)
    wk_d = inp("wk", [L, D, SH], BF16)
    wv_d = inp("wv", [L, D, SH], BF16)
    wo_d = inp("wo", [L, SH, D], BF16)
    w1_d = inp("w1", [L, D, FF], BF16)
    w2_d = inp("w2", [L, FF, D], BF16)
    bq_d = inp("bq", [L, 128, NQ])
    bk_d = inp("bk", [L, 128, NQ])
    bv_d = inp("bv", [L, 128, NQ])
    bo_d = inp("bo2", [L, 128, NT])     # pre-halved (RS sums over the pair)
    b1_d = inp("b1", [L, 128, NM])
    b2_d = inp("b2f", [L, 128, NT])     # full (no collective after w2)
    g1_d = inp("g1", [L, 128, NT])
    be1_d = inp("be1", [L, 128, NT])
    g2_d = inp("g2", [L, 128, NT])
    be2_d = inp("be2", [L, 128, NT])
    gf_d = inp("gf", [128, NT])
    bef_d = inp("bef", [128, NT])
    mask_d = inp("masks", [128, 4, CH])
    ones_d = inp("cones", [128, 128])

    out_ext = nc.dram_tensor("outt", [D, CH], F32, kind="ExternalOutput")
    out_v = out_ext.ap().rearrange("(t p) s -> p t s", p=128)

    with tile.TileContext(nc) as tc:
        with (
            nc.allow_low_precision(reason="bf16 matmuls + bf16 collectives"),
            tc.tile_pool(name="singles", bufs=1) as singles,
            tc.tile_pool(name="acts", bufs=1) as acts,
            tc.tile_pool(name="big", bufs=1) as big,
            tc.tile_pool(name="qkvw", bufs=1) as qkvw,
            tc.tile_pool(name="w1s", bufs=2) as w1s,
            tc.tile_pool(name="w2s", bufs=2) as w2s,
            tc.tile_pool(name="wt", bufs=8) as wtp,
            tc.tile_pool(name="rows", bufs=2) as rows,
            tc.tile_pool(name="evac", bufs=3) as evac,
            tc.tile_pool(name="pp", bufs=3, space="PSUM") as pp,
            tc.tile_pool(name="pav", bufs=2, space="PSUM") as pav,
            tc.tile_pool(name="prow", bufs=1, space="PSUM") as prow,
            tc.tile_pool(name="dram", bufs=2, space="DRAM") as dram,
        ):
            # ---- resident constants / state -----------------------------
            xT = singles.tile([128, NT, CH], F32R)
            nc.sync.dma_start(
                out=xT[:],
                in_=x0t.ap().rearrange("(t p) s -> p t s", p=128).bitcast(F32R))
            masks = singles.tile([128, 4, CH], BF16)
            nc.gpsimd.dma_start(out=masks[:], in_=mask_d.ap())
            onesr = singles.tile([128, 2], F32R)
            nc.sync.dma_start(out=onesr[:], in_=ones_d.ap()[:, 0:2].bitcast(F32R))
            onesb = singles.tile([1, 128], F32R)
            nc.sync.dma_start(out=onesb[:], in_=ones_d.ap()[0:1, :].bitcast(F32R))
            ones_kb_t = singles.tile([128, 1], BF16)
            nc.vector.memset(ones_kb_t[:], 1.0)
            ones_bcb_t = singles.tile([1, 128], BF16)
            nc.vector.memset(ones_bcb_t[:], 1.0)
            eps_t = singles.tile([1, 1], F32)
            nc.vector.memset(eps_t[:], EPS)
            invD_t = singles.tile([1, 1], F32)
            nc.vector.memset(invD_t[:], 1.0 / D)

            # V stripes [128, NT, 8*65]: per head 64 value cols + ones col
            # (softmax denominator rides the AV matmul). Ones written once.
            Vt = singles.tile([128, NT, HL * 65], BF16)
            nc.gpsimd.dma_start(
                out=Vt[:].rearrange("p t (h c) -> p t h c", h=HL)[:, :, :, 64:65],
                in_=ones_d.ap()[:, 0 : NT * HL]
                    .rearrange("p (t h o) -> p t h o", t=NT, h=HL))

            def load_pp(d, shape):
                t = singles.tile(list(shape), F32, name=f"pp_{d.name}")
                src = d.ap()
                if len(shape) == 3:
                    src = src.rearrange("l p m -> p l m")
                nc.sync.dma_start(out=t[:], in_=src)
                return t

            bqT = load_pp(bq_d, [128, L, NQ])
            bkT = load_pp(bk_d, [128, L, NQ])
            bvT = load_pp(bv_d, [128, L, NQ])
            boT = load_pp(bo_d, [128, L, NT])
            b1T = load_pp(b1_d, [128, L, NM])
            b2T = load_pp(b2_d, [128, L, NT])
            g1T = load_pp(g1_d, [128, L, NT])
            be1T = load_pp(be1_d, [128, L, NT])
            g2T = load_pp(g2_d, [128, L, NT])
            be2T = load_pp(be2_d, [128, L, NT])
            gfT = load_pp(gf_d, [128, NT])
            befT = load_pp(bef_d, [128, NT])

            ones_k = onesr[:, 0:1]            # [128,1] lhsT for column sums
            ones_kb = ones_kb_t[:, 0:1]       # [128,1] bf16 lhsT for sums
            ones_bc = onesb[:, :]             # [1,128] lhsT for broadcasts
            ones_bcb = ones_bcb_t[:, :]       # [1,128] bf16 lhsT broadcasts

            # ---- layernorm over the feature dim (512 tokens) ------------
            def ln_rowstats(psx, psx2, out01):
                """psx/psx2 PSUM sum rows -> out01 [2,CH] = [1/sigma, m/sigma].

                Chain kept on DVE (one Act hop for sqrt) to minimize
                cross-engine semaphore latency.
                """
                mrow = rows.tile([1, CH], F32R, tag="mr")
                nc.vector.tensor_scalar_mul(out=mrow[:], in0=psx[:],
                                            scalar1=invD_t[:])
                m2row = rows.tile([1, CH], F32R, tag="rb")
                nc.vector.tensor_tensor(out=m2row[:], in0=mrow[:],
                                        in1=mrow[:], op=MULT)
                vrow = rows.tile([1, CH], F32R, tag="rb")
                nc.vector.tensor_scalar_mul(out=vrow[:], in0=psx2[:],
                                            scalar1=invD_t[:])
                nc.vector.tensor_tensor(out=vrow[:], in0=vrow[:],
                                        in1=m2row[:], op=SUB)
                srow = rows.tile([1, CH], F32R, tag="rb")
                nc.scalar.activation(out=srow[:], in_=vrow[:],
                                     func=AF.Sqrt, bias=eps_t[:], scale=1.0)
                rr, mrr = out01
                nc.vector.reciprocal(out=rr[:], in_=srow[:])
                nc.vector.tensor_tensor(out=mrr[:], in0=mrow[:],
                                        in1=rr[:], op=MULT)

            def ln_norm(st01, gT, bT, l_idx, dest_fn, src, bdt=BF16):
                """normalize src tiles with stats rows (rr, mrr) -> dest_fn."""
                rr, mrr = st01
                blhs = ones_bcb if rr.dtype == BF16 else ones_bc
                prbP = prow.tile([128, CH], F32, tag="bc", bufs=1)
                nc.tensor.matmul(prbP[:], blhs, rr[:],
                                 start=True, stop=True)
                prb = evac.tile([128, CH], bdt, tag="rbb", bufs=1)
                nc.scalar.activation(out=prb[:], in_=prbP[:],
                                     func=AF.Identity, scale=1.0)
                pmrbP = prow.tile([128, CH], F32, tag="bc", bufs=1)
                nc.tensor.matmul(pmrbP[:], blhs, mrr[:],
                                 start=True, stop=True)
                pmrb = evac.tile([128, CH], bdt, tag="mrbb", bufs=1)
                nc.scalar.activation(out=pmrb[:], in_=pmrbP[:],
                                     func=AF.Identity, scale=1.0)
                for t in range(NT):
                    tmp = evac.tile([128, CH], bdt, tag="lntmp", bufs=3)
                    nc.vector.tensor_tensor(out=tmp[:], in0=src[:, t, :],
                                            in1=prb[:], op=MULT)
                    nc.vector.tensor_tensor(out=tmp[:], in0=tmp[:],
                                            in1=pmrb[:], op=SUB)
                    if l_idx is not None:
                        gs = gT[:, l_idx, t : t + 1]
                        bs = bT[:, l_idx, t : t + 1]
                    else:
                        gs = gT[:, t : t + 1]
                        bs = bT[:, t : t + 1]
                    dest_fn(t, tmp, gs, bs)

            def layernorm(gT, bT, l_idx, dest_fn, src=None, bdt=BF16):
                if src is None:
                    src = xT
                sum_lhs = ones_kb if src.dtype == BF16 else ones_k
                psx = prow.tile([1, CH], F32, tag="psx")
                psx2 = prow.tile([1, CH], F32, tag="psx2")
                for t in range(NT):
                    nc.tensor.matmul(psx[:], sum_lhs, src[:, t, :],
                                     start=(t == 0), stop=(t == NT - 1))
                for t in range(NT):
                    sq = evac.tile([128, CH], BF16, tag="lntmp", bufs=3)
                    nc.scalar.activation(out=sq[:], in_=src[:, t, :],
                                         func=AF.Square, scale=1.0)
                    nc.tensor.matmul(psx2[:], ones_kb, sq[:],
                                     start=(t == 0), stop=(t == NT - 1))
                rr = rows.tile([1, CH], F32R, tag="rrow")
                mrr = rows.tile([1, CH], F32R, tag="mrow2", bufs=1)
                ln_rowstats(psx, psx2, (rr, mrr))
                ln_norm((rr, mrr), gT, bT, l_idx, dest_fn, src, bdt=bdt)

            def load_w(dram_t, l, shape, rearr="(t p) m -> p t m", tag=None,
                       pool=qkvw, bufs=None, cols=None):
                t = pool.tile(list(shape), BF16, tag=tag or dram_t.name,
                              **({"bufs": bufs} if bufs else {}))
                src = dram_t.ap()[l]
                if cols is not None:
                    src = src[:, cols]
                nc.sync.dma_start(out=t[:], in_=src.rearrange(rearr, p=128))
                return t

            # Cross-pair x exchange for the next layer's LN1+QKV, in two
            # pieces: (1) post-attention x (bf16), gathered during the MLP
            # (fully hidden); (2) only the MLP delta (fp8), gathered in two
            # feature halves right as the w2 m-tiles finish, so just ~28us
            # of the second half is exposed. The fp8 half also carries the
            # next layer's LN1 row stats (computed from own f32 x) as a
            # bf16-bitcast tail, so the receiver skips the stats pass.
            XQT = 128 * CH                     # elems per feature tile
            FTA = 4 * XQT                      # fp8 delta elems, first AG
            FTB = 4 * XQT                      # fp8 delta elems, second AG
            FST = 4 * CH                       # stats tail (2xCH bf16 bytes)

            def ffd_copy(gin, t, po):
                pof = evac.tile([128, CH], F8, tag="po", bufs=2)
                nc.scalar.activation(out=pof[:], in_=po[:],
                                     func=AF.Identity, scale=1.0)
                o = (t % 4) * XQT
                nc.sync.dma_start(
                    out=gin[o:o + XQT].rearrange("(p q) -> p q", p=128),
                    in_=pof[:])

            def gather_go(gin, lnext, half):
                n = (FTB + FST) if half == 1 else FTA
                gout = dram.tile([2, n], F8, tag=f"agxo{half}",
                                 name=f"agxo{lnext}_{half}")
                nc.gpsimd.collective_compute(
                    "AllGather", BYPASS, replica_groups=REPLICA_GROUPS,
                    ins=[gin.opt()], outs=[gout.opt()])
                return gout

            # ---- main body ----------------------------------------------
            agx_prev = None
            for l in range(L):
                # LN1 over both token chunks from the gathered (bf16) x;
                # identical h on both pair members, no h collective.
                hT = big.tile([128, NT, S], BF16, tag="big", name=f"hT{l}")
                for c in range(NCH):
                    xp = acts.tile([128, NT, CH], BF16, tag="h2", bufs=2,
                                   name=f"xp{l}_{c}")

                    def wr_hc(t, tmp, gs, bs, c=c, hT=hT):
                        nc.scalar.activation(
                            out=hT[:, t, c * CH:(c + 1) * CH], in_=tmp[:],
                            func=AF.Identity, scale=gs, bias=bs)

                    if l == 0:
                        nc.sync.dma_start(out=xp[:], in_=x0f_d.ap()[c])
                        st_r = rows.tile([1, CH], BF16, tag="strowr",
                                         bufs=1, name=f"strowr{l}_{c}")
                        st_m = rows.tile([1, CH], BF16, tag="strowm",
                                         bufs=1, name=f"strowm{l}_{c}")
                        nc.sync.dma_start(out=st_r[:],
                                          in_=st0_d.ap()[c, 0:1, :])
                        nc.sync.dma_start(out=st_m[:],
                                          in_=st0_d.ap()[c, 1:2, :])
                        ln_norm((st_r, st_m), g1T, be1T, l, wr_hc, src=xp)
                    else:
                        gxp, ga, gb = agx_prev
                        nc.gpsimd.dma_start(
                            out=xp[:],
                            in_=gxp[c].rearrange("(t p q) -> p t q",
                                                 t=NT, p=128))
                        st_r = rows.tile([1, CH], BF16, tag="strowr",
                                         bufs=1, name=f"strowr{l}_{c}")
                        st_m = rows.tile([1, CH], BF16, tag="strowm",
                                         bufs=1, name=f"strowm{l}_{c}")
                        nc.gpsimd.dma_start(
                            out=st_r[:],
                            in_=gb[c, FTB:FTB + 2 * CH].bitcast(BF16)
                                .rearrange("(r q) -> r q", r=1))
                        nc.gpsimd.dma_start(
                            out=st_m[:],
                            in_=gb[c, FTB + 2 * CH:FTB + FST].bitcast(BF16)
                                .rearrange("(r q) -> r q", r=1))
                        for t in range(NT):
                            g = ga if t < 4 else gb
                            o = (t % 4) * XQT
                            fft = evac.tile([128, CH], F8, tag="po", bufs=2)
                            nc.gpsimd.dma_start(
                                out=fft[:],
                                in_=g[c, o:o + XQT].rearrange("(p q) -> p q",
                                                              p=128))
                            nc.vector.tensor_tensor(
                                out=xp[:, t, :], in0=xp[:, t, :],
                                in1=fft[:], op=ADD)
                        ln_norm((st_r, st_m), g1T, be1T, l, wr_hc, src=xp)

                # per-layer weight tiles (single DMAs, 1KB contiguous rows)
                wqL = load_w(wq_d, l, [128, NT, SH])
                wkL = load_w(wk_d, l, [128, NT, SH])
                wvL = load_w(wv_d, l, [128, NT, SH])
                woL = load_w(wo_d, l, [128, NQ, D])

                # K projection -> feature-major [128, NQ, S] bf16
                # (chunk-major so chunk 0 attention can start early)
                KT = acts.tile([128, NQ, S], BF16, tag="kt")
                for c in range(NCH):
                    for m in range(NQ):
                        cs = slice(c * CH, (c + 1) * CH)
                        ps = pp.tile([128, CH], F32, tag="pp")
                        for k in range(NT):
                            nc.tensor.matmul(
                                ps[:], wkL[:, k, m * 128:(m + 1) * 128],
                                hT[:, k, cs],
                                start=(k == 0), stop=(k == NT - 1))
                        nc.scalar.activation(
                            out=KT[:, m, cs], in_=ps[:], func=AF.Identity,
                            bias=bkT[:, l, m : m + 1], scale=1.0)

                # V projection -> token-major stripes (64 cols + ones col)
                def v_proj(mt):
                    ps = pp.tile([128, SH], F32, tag="pp")
                    for k in range(NT):
                        nc.tensor.matmul(
                            ps[:], hT[:, k, mt * 128:(mt + 1) * 128],
                            wvL[:, k, :],
                            start=(k == 0), stop=(k == NT - 1))
                    nc.vector.tensor_copy(
                        out=Vt[:, mt, :].rearrange("p (h c) -> p h c", h=HL)[:, :, 0:64],
                        in_=ps[:].rearrange("p (h c) -> p h c", h=HL))

                for mt in range(4):
                    v_proj(mt)

                # attention -> attnT [128, NQ, S] bf16, then wo partials
                attnT = acts.tile([128, NQ, S], BF16, tag="at")
                rs_in = dram.tile([NCH, 128, NT, CH], BF16, tag="rsin")
                rs_out = dram.tile([128, NT, CH], BF16, tag="rsout")
                for c in range(NCH):
                    cs = slice(c * CH, (c + 1) * CH)
                    nk = 4 * c + 4
                    QTc = acts.tile([128, NQ, CH], BF16, tag="qt", bufs=1)
                    for m in range(NQ):
                        ps = pp.tile([128, CH], F32, tag="pp")
                        for k in range(NT):
                            nc.tensor.matmul(
                                ps[:], wqL[:, k, m * 128:(m + 1) * 128],
                                hT[:, k, cs],
                                start=(k == 0), stop=(k == NT - 1))
                        nc.scalar.activation(
                            out=QTc[:, m, :], in_=ps[:], func=AF.Identity,
                            bias=bqT[:, l, m : m + 1], scale=1.0)
                    if c == 0:
                        for mt in range(4, NT):
                            v_proj(mt)
                    for h in range(HL):
                        base = 64 * (h % 2)
                        hp = h // 2
                        wts = []
                        for j in range(nk):
                            pl = pp.tile([128, CH], F32, tag="pp")
                            nc.tensor.matmul(
                                pl[:],
                                KT[base:base + 64, hp, j * 128:(j + 1) * 128],
                                QTc[base:base + 64, hp, :],
                                start=True, stop=True)
                            wt = wtp.tile([128, CH], BF16, tag="wt", bufs=8)
                            nc.scalar.activation(out=wt[:], in_=pl[:],
                                                 func=AF.Exp, scale=0.125)
                            r = j - 4 * c
                            if r >= 0:
                                nc.vector.tensor_tensor(
                                    out=wt[:], in0=wt[:],
                                    in1=masks[:, r, :], op=MULT)
                            wts.append(wt)
                        pa = pav.tile([65, CH], F32, tag="pav")
                        for j in range(nk):
                            nc.tensor.matmul(
                                pa[:], Vt[:, j, 65 * h : 65 * h + 65],
                                wts[j][:],
                                start=(j == 0), stop=(j == nk - 1))
                        rec = rows.tile([1, CH], F32R, tag="rb")
                        nc.vector.reciprocal(out=rec[:], in_=pa[64:65, :])
                        # broadcast 1/denominator across partitions via a
                        # K=1 ones-matmul into the shared bc PSUM bank, then
                        # to SBUF (DVE cannot read two PSUM operands)
                        pbc = prow.tile([64, CH], F32, tag="bc", bufs=1)
                        nc.tensor.matmul(pbc[:], ones_bc[:, 0:64],
                                         rec[:], start=True, stop=True)
                        recb = wtp.tile([64, CH], BF16, tag="recb", bufs=2)
                        nc.vector.tensor_copy(out=recb[:], in_=pbc[:])
                        nc.vector.tensor_tensor(
                            out=attnT[base:base + 64, hp, cs],
                            in0=pa[0:64, :], in1=recb[:], op=MULT)
                    for t in range(NQ):
                        nc.vector.tensor_scalar_add(
                            out=attnT[:, t, cs], in0=attnT[:, t, cs],
                            scalar1=bvT[:, l, t : t + 1])
                    # wo partials for this chunk -> ReduceScatter segment c
                    for m in range(NT):
                        ps = pp.tile([128, CH], F32, tag="pp")
                        for k in range(NQ):
                            nc.tensor.matmul(
                                ps[:], woL[:, k, m * 128:(m + 1) * 128],
                                attnT[:, k, cs],
                                start=(k == 0), stop=(k == NQ - 1))
                        po = evac.tile([128, CH], BF16, tag="po", bufs=2)
                        if c == 0 or m >= 6:
                            nc.scalar.activation(
                                out=po[:], in_=ps[:], func=AF.Identity,
                                bias=boT[:, l, m : m + 1], scale=1.0)
                        else:
                            nc.vector.tensor_scalar_add(
                                out=po[:], in0=ps[:],
                                scalar1=boT[:, l, m : m + 1])
                        nc.sync.dma_start(out=rs_in[c, :, m, :], in_=po[:])
                nc.gpsimd.collective_compute(
                    "ReduceScatter", ADD, replica_groups=REPLICA_GROUPS,
                    ins=[rs_in.opt()], outs=[rs_out.opt()])

                # residual(x) += RS result, then LN2 -> h2 (bf16).
                # Post-attention x is copied out per tile and AllGathered
                # during the MLP (fully hidden).
                rsS = acts.tile([128, NT, CH], BF16, tag="h2", bufs=2,
                                name=f"rsS{l}")
                nc.sync.dma_start(out=rsS[:, 0:4, :], in_=rs_out[:, 0:4, :])
                nc.sync.dma_start(out=rsS[:, 4:8, :], in_=rs_out[:, 4:8, :])
                for t in range(NT):
                    nc.vector.tensor_tensor(
                        out=xT[:, t, :], in0=xT[:, t, :],
                        in1=rsS[:, t, :], op=ADD)
                h2 = acts.tile([128, NT, CH], BF16, tag="h2", bufs=2)

                def wr_h2(t, tmp, gs, bs, h2=h2):
                    nc.vector.tensor_scalar(
                        out=h2[:, t, :], in0=tmp[:],
                        scalar1=gs, scalar2=bs, op0=MULT, op1=ADD)

                layernorm(g2T, be2T, l, wr_h2)

                # post-attention x copy-out + hidden AllGather (during MLP)
                if l + 1 < L:
                    gxp_in = dram.tile([NT * XQT], BF16, tag="gxpi",
                                       name=f"gxpi{l + 1}")
                    for t in range(NT):
                        xc = evac.tile([128, CH], BF16, tag="po", bufs=2)
                        nc.scalar.activation(out=xc[:], in_=xT[:, t, :],
                                             func=AF.Identity, scale=1.0)
                        nc.sync.dma_start(
                            out=gxp_in[t * XQT:(t + 1) * XQT]
                                .rearrange("(p q) -> p q", p=128),
                            in_=xc[:])
                    gxp_out = dram.tile([2, NT * XQT], BF16, tag="gxpo",
                                        name=f"gxpo{l + 1}")
                    nc.gpsimd.collective_compute(
                        "AllGather", BYPASS, replica_groups=REPLICA_GROUPS,
                        ins=[gxp_in.opt()], outs=[gxp_out.opt()])

                # MLP: full 4096-wide FF over own 512 tokens, no collective
                gT = big.tile([128, NM, CH], BF16, tag="big", name=f"gT{l}")
                for mm in range(16):
                    w1t = w1s.tile([128, NT, 256], BF16, tag="w1")
                    nc.sync.dma_start(
                        out=w1t[:],
                        in_=w1_d.ap()[l][:, mm * 256:(mm + 1) * 256]
                            .rearrange("(t p) m -> p t m", p=128))
                    for ms in range(2):
                        m = mm * 2 + ms
                        ps = pp.tile([128, CH], F32, tag="pp")
                        for k in range(NT):
                            nc.tensor.matmul(
                                ps[:], w1t[:, k, ms * 128:(ms + 1) * 128],
                                h2[:, k, :],
                                start=(k == 0), stop=(k == NT - 1))
                        nc.scalar.activation(
                            out=gT[:, m, :], in_=ps[:], func=AF.Gelu,
                            bias=b1T[:, l, m : m + 1], scale=1.0)
                agx_a = None
                if l + 1 < L:
                    gin_a = dram.tile([FTA], F8, tag="agxi0",
                                      name=f"agxi{l + 1}_0")
                    gin_b = dram.tile([FTB + FST], F8, tag="agxi1",
                                      name=f"agxi{l + 1}_1")
                    psxN = prow.tile([1, CH], F32, tag="psx")
                    psx2N = prow.tile([1, CH], F32, tag="psx2")
                for m2 in range(4):
                    w2t = w2s.tile([128, NM, 256], BF16, tag="w2")
                    nc.sync.dma_start(
                        out=w2t[:],
                        in_=w2_d.ap()[l][:, m2 * 256:(m2 + 1) * 256]
                            .rearrange("(t p) m -> p t m", p=128))
                    for ms in range(2):
                        m = m2 * 2 + ms
                        ps = pp.tile([128, CH], F32, tag="pp")
                        for k in range(NM):
                            nc.tensor.matmul(
                                ps[:], w2t[:, k, ms * 128:(ms + 1) * 128],
                                gT[:, k, :],
                                start=(k == 0), stop=(k == NM - 1))
                        po = evac.tile([128, CH], BF16, tag="po", bufs=2)
                        nc.scalar.activation(
                            out=po[:], in_=ps[:], func=AF.Identity,
                            bias=b2T[:, l, m : m + 1], scale=1.0)
                        nc.vector.tensor_tensor(
                            out=xT[:, m, :], in0=xT[:, m, :],
                            in1=po[:], op=ADD)
                        if l + 1 < L:
                            ffd_copy(gin_a if m < 4 else gin_b, m, po)
                            nc.tensor.matmul(psxN[:], ones_k, xT[:, m, :],
                                             start=(m == 0),
                                             stop=(m == NT - 1))
                            sq = evac.tile([128, CH], BF16, tag="lntmp",
                                           bufs=3)
                            nc.vector.tensor_tensor(out=sq[:],
                                                    in0=xT[:, m, :],
                                                    in1=xT[:, m, :],
                                                    op=MULT)
                            nc.tensor.matmul(psx2N[:], ones_kb, sq[:],
                                             start=(m == 0),
                                             stop=(m == NT - 1))
                            if m == 3:
                                agx_a = gather_go(gin_a, l + 1, 0)
                if l + 1 < L:
                    stb_r = rows.tile([1, CH], BF16, tag="stbr", bufs=1,
                                      name=f"stbr{l + 1}")
                    stb_m = rows.tile([1, CH], BF16, tag="stbm", bufs=1,
                                      name=f"stbm{l + 1}")
                    ln_rowstats(psxN, psx2N, (stb_r, stb_m))
                    nc.sync.dma_start(
                        out=gin_b[FTB:FTB + 2 * CH].bitcast(BF16)
                            .rearrange("(r q) -> r q", r=1),
                        in_=stb_r[:])
                    nc.sync.dma_start(
                        out=gin_b[FTB + 2 * CH:FTB + FST].bitcast(BF16)
                            .rearrange("(r q) -> r q", r=1),
                        in_=stb_m[:])
                    agx_prev = (gxp_out, agx_a,
                                gather_go(gin_b, l + 1, 1))

            # final LN -> output (own token half)
            def wr_out(t, tmp, gs, bs):
                ot = evac.tile([128, CH], F32, tag="ot", bufs=1)
                nc.vector.tensor_scalar(out=ot[:], in0=tmp[:],
                                        scalar1=gs, scalar2=bs,
                                        op0=MULT, op1=ADD)
                nc.sync.dma_start(out=out_v[:, t, :], in_=ot[:])

            layernorm(gfT, befT, None, wr_out)

    return nc


# ---------------------------------------------------------------------------
# host side
# ---------------------------------------------------------------------------

def _sinusoidal_pe(s, d):
    pos = np.arange(s, dtype=np.float32)[:, None]
    div = np.exp(np.arange(0, d, 2, dtype=np.float32)
                 * np.float32(-np.log(10000.0) / d)).astype(np.float32)
    pe = np.zeros((s, d), dtype=np.float32)
    pe[:, 0::2] = np.sin(pos * div)
    pe[:, 1::2] = np.cos(pos * div)
    return pe


def _pp128(v):
    """[L?, n*128] -> [L?, 128, n] with feature = 128*m + p."""
    v = np.asarray(v, dtype=np.float32)
    if v.ndim == 1:
        return np.ascontiguousarray(v.reshape(-1, 128).T)
    lq, n = v.shape
    return np.ascontiguousarray(v.reshape(lq, n // 128, 128).transpose(0, 2, 1))


_NC_CACHE = {}


def _get_nc(repeat=1):
    if repeat not in _NC_CACHE:
        _NC_CACHE[repeat] = build_nc()
    return _NC_CACHE[repeat]


def make_in_maps(input_ids, tok_emb, wq, bq, wk, bk, wv, bv, wo, bo,
                 ln1_g, ln1_b, ln2_g, ln2_b, w1, b1, w2, b2, lnf_g, lnf_b):
    input_ids = np.asarray(input_ids)
    pe = _sinusoidal_pe(S, D)
    masks = np.zeros((128, 4, CH), dtype=np.float32)
    ar = np.arange(CH)
    for r in range(4):
        for p in range(128):
            masks[p, r, :] = (ar >= 128 * r + p).astype(np.float32)
    cones = np.ones((128, 128), dtype=np.float32)
    bf = ml_dtypes.bfloat16

    in_maps = []
    for core in range(N_CORES):
        b = core // 2
        j = core % 2
        qs = slice(j * SH, (j + 1) * SH)
        ts = slice(j * CH, (j + 1) * CH)
        x0 = (tok_emb[input_ids[b]] + pe).astype(np.float32)   # [S, D]
        x0f = np.ascontiguousarray(
            x0.T.reshape(NT, 128, NCH, CH).transpose(2, 1, 0, 3)).astype(bf)
        m0 = x0.mean(axis=1)
        r0 = 1.0 / np.sqrt(x0.var(axis=1) + 1e-5)
        st0 = np.stack([r0, m0 * r0]).astype(np.float32)       # [2, S]
        st0 = np.ascontiguousarray(
            st0.reshape(2, NCH, CH).transpose(1, 0, 2)).astype(bf)
        m = {
            "x0t": np.ascontiguousarray(x0.T[:, ts]),
            "x0f": x0f,
            "st0": st0,
            "wq": np.ascontiguousarray(wq[:, :, qs]).astype(bf),
            "wk": np.ascontiguousarray(wk[:, :, qs]).astype(bf),
            "wv": np.ascontiguousarray(wv[:, :, qs]).astype(bf),
            "wo": np.ascontiguousarray(wo[:, qs, :]).astype(bf),
            "w1": np.ascontiguousarray(w1).astype(bf),
            "w2": np.ascontiguousarray(w2).astype(bf),
            "bq": _pp128(bq[:, qs]),
            "bk": _pp128(bk[:, qs]),
            "bv": _pp128(bv[:, qs]),
            "bo2": _pp128(bo * 0.5),
            "b1": _pp128(b1),
            "b2f": _pp128(b2),
            "g1": _pp128(ln1_g),
            "be1": _pp128(ln1_b),
            "g2": _pp128(ln2_g),
            "be2": _pp128(ln2_b),
            "gf": _pp128(lnf_g),
            "bef": _pp128(lnf_b),
            "masks": masks,
            "cones": cones,
        }
        in_maps.append(m)
    return in_maps


def kernel(input_ids, attention_mask, tok_emb, ln1_g, ln1_b, wq, bq, wk, bk,
           wv, bv, wo, bo, ln2_g, ln2_b, w1, b1, w2, b2, lnf_g, lnf_b,
           _repeat=1):
    args = [np.asarray(a, dtype=np.float32) for a in
            (tok_emb, wq, bq, wk, bk, wv, bv, wo, bo,
             ln1_g, ln1_b, ln2_g, ln2_b, w1, b1, w2, b2, lnf_g, lnf_b)]
    (tok_emb, wq, bq, wk, bk, wv, bv, wo, bo,
     ln1_g, ln1_b, ln2_g, ln2_b, w1, b1, w2, b2, lnf_g, lnf_b) = args
    in_maps = make_in_maps(input_ids, tok_emb, wq, bq, wk, bk, wv, bv, wo, bo,
                           ln1_g, ln1_b, ln2_g, ln2_b, w1, b1, w2, b2,
                           lnf_g, lnf_b)
    nc = _get_nc(_repeat)
    res = run_bass_kernel_spmd(nc, in_maps, list(range(N_CORES)))
    out = np.empty((B, S, D), dtype=np.float32)
    for b in range(B):
        for j in range(2):
            out[b, j * CH:(j + 1) * CH] = res.results[2 * b + j]["outt"].T
    return out


# revision 94
# speedup vs baseline: 1.0151x; 1.0031x over previous
"""Trainium2 Bass kernel for nn_DecoderLM_91018946936840.

4-layer pre-LN decoder (D=1024, H=16, S=1024, B=4, ff=4096) on 8 NeuronCores:
data-parallel over B (4 pair-groups) x Megatron-SP tensor-parallel 2 within
each pair: attention is head-sharded (8 heads/core), LayerNorm / residual /
MLP are sequence-sharded (512 tokens/core, full 4096-wide FF, no collective).
Per layer: one AllGather of the LN1 output (bf16) before QKV and one
ReduceScatter (add) after the attention out-projection.

Activations are feature-major [D, tokens]: LayerNorm statistics come from
ones-matmuls, per-token scalars are broadcast across partitions via small
DRAM round-trip DMAs, and each head's softmax denominator rides along the AV
matmul as a ones-column appended to V. Matmuls run bf16 with fp32 PSUM.
"""
import numpy as np
import ml_dtypes

import concourse.bass as bass
import concourse.mybir as mybir
import concourse.tile as tile
from concourse.bass_utils import run_bass_kernel_spmd
from concourse.vector_clock import ScopedClock

# ---------------------------------------------------------------------------
# Workaround: this walrus build accepts at most ONE semaphore wait per
# instruction ("Too many sync wait commands"). Redistribute Tile-assigned
# waits onto single-wait NoOps in front of the owning instruction, and do the
# same for the kernel-tail drain.
# ---------------------------------------------------------------------------
_MAX_WAITS = 1


def _patched_drain_and_barrier(self, tick_clock, wait_clock):
    nc = self.nc
    probe = nc.sync.nop(hint="drain_waits", nofuse=True)
    wait_clock.add_sem_waits(probe.ins, ScopedClock({None: tick_clock.global_clock}))
    si = probe.ins.sync_info
    waits = list(si.on_wait) if si is not None else []
    probe.ins.sync_info = mybir.SyncInfo(
        on_wait=waits[:_MAX_WAITS],
        on_update=list(si.on_update) if si is not None else [],
    )
    for i in range(_MAX_WAITS, len(waits), _MAX_WAITS):
        extra = nc.sync.nop(hint="drain_waits", nofuse=True)
        extra.ins.sync_info = mybir.SyncInfo(
            on_wait=waits[i : i + _MAX_WAITS], on_update=[])
    nc.sync.drain()
    nc.all_engine_barrier()
    assert self.sems is not None
    popped = nc._tile_sem_poison_stack.pop()
    assert popped is self._sem_poison
    nc.clear_and_free_semaphores(list(self.sems.allocated().values()))
    nc.all_engine_barrier()


_orig_commit = tile.TileContext._commit_instruction


def _patched_commit_instruction(self, inst, lazy_reg_writes=True):
    si = inst.sync_info
    if si is not None and len(si.on_wait) > _MAX_WAITS:
        waits = list(si.on_wait)
        keep, extras = waits[-_MAX_WAITS:], waits[:-_MAX_WAITS]
        engine = inst.engine
        if engine == mybir.EngineType.Unassigned:
            engine = mybir.EngineType.SP
        for w in extras:
            nop = mybir.InstNoOp(
                name=self.nc.get_next_instruction_name(),
                ins=[],
                outs=[],
                engine=engine,
                sync_info=mybir.SyncInfo(on_wait=[w], on_update=[]),
            )
            self._add_instruction(nop)
        inst.sync_info = mybir.SyncInfo(on_wait=keep, on_update=list(si.on_update))
    return _orig_commit(self, inst, lazy_reg_writes)


tile.TileContext._drain_and_barrier = _patched_drain_and_barrier
tile.TileContext._commit_instruction = _patched_commit_instruction

# ---------------------------------------------------------------------------

V, D, H, L, B, S = 32000, 1024, 16, 4, 4, 1024
HD = D // H          # 64
FF = 4 * D           # 4096
EPS = 1e-5
N_CORES = 8
SH = D // 2          # 512   qkv output shard per core (8 heads)
NT = D // 128        # 8     model-dim tiles
NQ = SH // 128       # 4     shard-dim tiles
NM = FF // 128       # 32    full-ffn m-tiles
CH = 512             # token chunk == own token half
NCH = S // CH        # 2
HL = 8               # heads per core

F32 = mybir.dt.float32
F32R = mybir.dt.float32r
BF16 = mybir.dt.bfloat16
ADD = mybir.AluOpType.add
MULT = mybir.AluOpType.mult
SUB = mybir.AluOpType.subtract
BYPASS = mybir.AluOpType.bypass
F8 = mybir.dt.float8e4
AF = mybir.ActivationFunctionType

REPLICA_GROUPS = [[0, 1], [2, 3], [4, 5], [6, 7]]


def build_nc():
    nc = bass.Bass(trn_type="TRN2", target_bir_lowering=False, debug=False,
                   num_devices=N_CORES)

    def inp(name, shape, dt=F32):
        return nc.dram_tensor(name, list(shape), dt, kind="ExternalInput")

    x0t = inp("x0t", [D, CH])
    x0f_d = inp("x0f", [NCH, 128, NT, CH], BF16)
    st0_d = inp("st0", [NCH, 2, CH], BF16)
    wq_d = inp("wq", [L, D, SH], BF16)
    wk_d = inp("wk", [L, D, SH], BF16)
    wv_d = inp("wv", [L, D, SH], BF16)
    wo_d = inp("wo", [L, SH, D], BF16)
    w1_d = inp("w1", [L, D, FF], BF16)
    w2_d = inp("w2", [L, FF, D], BF16)
    bq_d = inp("bq", [L, 128, NQ])
    bk_d = inp("bk", [L, 128, NQ])
    bv_d = inp("bv", [L, 128, NQ])
    bo_d = inp("bo2", [L, 128, NT])     # pre-halved (RS sums over the pair)
    b1_d = inp("b1", [L, 128, NM])
    b2_d = inp("b2f", [L, 128, NT])     # full (no collective after w2)
    g1_d = inp("g1", [L, 128, NT])
    be1_d = inp("be1", [L, 128, NT])
    g2_d = inp("g2", [L, 128, NT])
    be2_d = inp("be2", [L, 128, NT])
    gf_d = inp("gf", [128, NT])
    bef_d = inp("bef", [128, NT])
    mask_d = inp("masks", [128, 4, CH])
    ones_d = inp("cones", [128, 128])

    out_ext = nc.dram_tensor("outt", [D, CH], F32, kind="ExternalOutput")
    out_v = out_ext.ap().rearrange("(t p) s -> p t s", p=128)

    with tile.TileContext(nc) as tc:
        with (
            nc.allow_low_precision(reason="bf16 matmuls + bf16 collectives"),
            tc.tile_pool(name="singles", bufs=1) as singles,
            tc.tile_pool(name="acts", bufs=1) as acts,
            tc.tile_pool(name="big", bufs=1) as big,
            tc.tile_pool(name="qkvw", bufs=1) as qkvw,
            tc.tile_pool(name="w1s", bufs=2) as w1s,
            tc.tile_pool(name="w2s", bufs=2) as w2s,
            tc.tile_pool(name="wt", bufs=8) as wtp,
            tc.tile_pool(name="rows", bufs=2) as rows,
            tc.tile_pool(name="evac", bufs=3) as evac,
            tc.tile_pool(name="pp", bufs=3, space="PSUM") as pp,
            tc.tile_pool(name="pav", bufs=2, space="PSUM") as pav,
            tc.tile_pool(name="prow", bufs=1, space="PSUM") as prow,
            tc.tile_pool(name="dram", bufs=2, space="DRAM") as dram,
        ):
            # ---- resident constants / state -----------------------------
            xT = singles.tile([128, NT, CH], F32R)
            nc.sync.dma_start(
                out=xT[:],
                in_=x0t.ap().rearrange("(t p) s -> p t s", p=128).bitcast(F32R))
            masks = singles.tile([128, 4, CH], BF16)
            nc.gpsimd.dma_start(out=masks[:], in_=mask_d.ap())
            onesr = singles.tile([128, 2], F32R)
            nc.sync.dma_start(out=onesr[:], in_=ones_d.ap()[:, 0:2].bitcast(F32R))
            onesb = singles.tile([1, 128], F32R)
            nc.sync.dma_start(out=onesb[:], in_=ones_d.ap()[0:1, :].bitcast(F32R))
            ones_kb_t = singles.tile([128, 1], BF16)
            nc.vector.memset(ones_kb_t[:], 1.0)
            ones_bcb_t = singles.tile([1, 128], BF16)
            nc.vector.memset(ones_bcb_t[:], 1.0)
            eps_t = singles.tile([1, 1], F32)
            nc.vector.memset(eps_t[:], EPS)
            invD_t = singles.tile([1, 1], F32)
            nc.vector.memset(invD_t[:], 1.0 / D)

            # V stripes [128, NT, 8*65]: per head 64 value cols + ones col
            # (softmax denominator rides the AV matmul). Ones written once.
            Vt = singles.tile([128, NT, HL * 65], BF16)
            nc.gpsimd.dma_start(
                out=Vt[:].rearrange("p t (h c) -> p t h c", h=HL)[:, :, :, 64:65],
                in_=ones_d.ap()[:, 0 : NT * HL]
                    .rearrange("p (t h o) -> p t h o", t=NT, h=HL))

            def load_pp(d, shape):
                t = singles.tile(list(shape), F32, name=f"pp_{d.name}")
                src = d.ap()
                if len(shape) == 3:
                    src = src.rearrange("l p m -> p l m")
                nc.sync.dma_start(out=t[:], in_=src)
                return t

            bqT = load_pp(bq_d, [128, L, NQ])
            bkT = load_pp(bk_d, [128, L, NQ])
            bvT = load_pp(bv_d, [128, L, NQ])
            boT = load_pp(bo_d, [128, L, NT])
            b1T = load_pp(b1_d, [128, L, NM])
            b2T = load_pp(b2_d, [128, L, NT])
            g1T = load_pp(g1_d, [128, L, NT])
            be1T = load_pp(be1_d, [128, L, NT])
            g2T = load_pp(g2_d, [128, L, NT])
            be2T = load_pp(be2_d, [128, L, NT])
            gfT = load_pp(gf_d, [128, NT])
            befT = load_pp(bef_d, [128, NT])

            ones_k = onesr[:, 0:1]            # [128,1] lhsT for column sums
            ones_kb = ones_kb_t[:, 0:1]       # [128,1] bf16 lhsT for sums
            ones_bc = onesb[:, :]             # [1,128] lhsT for broadcasts
            ones_bcb = ones_bcb_t[:, :]       # [1,128] bf16 lhsT broadcasts

            # ---- layernorm over the feature dim (512 tokens) ------------
            def ln_rowstats(psx, psx2, out01):
                """psx/psx2 PSUM sum rows -> out01 [2,CH] = [1/sigma, m/sigma].

                Chain kept on DVE (one Act hop for sqrt) to minimize
                cross-engine semaphore latency.
                """
                mrow = rows.tile([1, CH], F32R, tag="mr")
                nc.vector.tensor_scalar_mul(out=mrow[:], in0=psx[:],
                                            scalar1=invD_t[:])
                m2row = rows.tile([1, CH], F32R, tag="rb")
                nc.vector.tensor_tensor(out=m2row[:], in0=mrow[:],
                                        in1=mrow[:], op=MULT)
                vrow = rows.tile([1, CH], F32R, tag="rb")
                nc.vector.tensor_scalar_mul(out=vrow[:], in0=psx2[:],
                                            scalar1=invD_t[:])
                nc.vector.tensor_tensor(out=vrow[:], in0=vrow[:],
                                        in1=m2row[:], op=SUB)
                srow = rows.tile([1, CH], F32R, tag="rb")
                nc.scalar.activation(out=srow[:], in_=vrow[:],
                                     func=AF.Sqrt, bias=eps_t[:], scale=1.0)
                rr, mrr = out01
                nc.vector.reciprocal(out=rr[:], in_=srow[:])
                nc.vector.tensor_tensor(out=mrr[:], in0=mrow[:],
                                        in1=rr[:], op=MULT)

            def ln_norm(st01, gT, bT, l_idx, dest_fn, src, bdt=BF16):
                """normalize src tiles with stats rows (rr, mrr) -> dest_fn."""
                rr, mrr = st01
                blhs = ones_bcb if rr.dtype == BF16 else ones_bc
                prbP = prow.tile([128, CH], F32, tag="bc", bufs=1)
                nc.tensor.matmul(prbP[:], blhs, rr[:],
                                 start=True, stop=True)
                prb = evac.tile([128, CH], bdt, tag="rbb", bufs=1)
                nc.vector.tensor_copy(out=prb[:], in_=prbP[:])
                pmrbP = prow.tile([128, CH], F32, tag="bc", bufs=1)
                nc.tensor.matmul(pmrbP[:], blhs, mrr[:],
                                 start=True, stop=True)
                pmrb = evac.tile([128, CH], bdt, tag="mrbb", bufs=1)
                nc.vector.tensor_copy(out=pmrb[:], in_=pmrbP[:])
                for t in range(NT):
                    tmp = evac.tile([128, CH], bdt, tag="lntmp", bufs=3)
                    nc.vector.tensor_tensor(out=tmp[:], in0=src[:, t, :],
                                            in1=prb[:], op=MULT)
                    nc.vector.tensor_tensor(out=tmp[:], in0=tmp[:],
                                            in1=pmrb[:], op=SUB)
                    if l_idx is not None:
                        gs = gT[:, l_idx, t : t + 1]
                        bs = bT[:, l_idx, t : t + 1]
                    else:
                        gs = gT[:, t : t + 1]
                        bs = bT[:, t : t + 1]
                    dest_fn(t, tmp, gs, bs)

            def layernorm(gT, bT, l_idx, dest_fn, src=None, bdt=BF16):
                if src is None:
                    src = xT
                sum_lhs = ones_kb if src.dtype == BF16 else ones_k
                psx = prow.tile([1, CH], F32, tag="psx")
                psx2 = prow.tile([1, CH], F32, tag="psx2")
                for t in range(NT):
                    nc.tensor.matmul(psx[:], sum_lhs, src[:, t, :],
                                     start=(t == 0), stop=(t == NT - 1))
                for t in range(NT):
                    sq = evac.tile([128, CH], BF16, tag="lntmp", bufs=3)
                    nc.scalar.activation(out=sq[:], in_=src[:, t, :],
                                         func=AF.Square, scale=1.0)
                    nc.tensor.matmul(psx2[:], ones_kb, sq[:],
                                     start=(t == 0), stop=(t == NT - 1))
                rr = rows.tile([1, CH], F32R, tag="rrow")
                mrr = rows.tile([1, CH], F32R, tag="mrow2", bufs=1)
                ln_rowstats(psx, psx2, (rr, mrr))
                ln_norm((rr, mrr), gT, bT, l_idx, dest_fn, src, bdt=bdt)

            def load_w(dram_t, l, shape, rearr="(t p) m -> p t m", tag=None,
                       pool=qkvw, bufs=None, cols=None):
                t = pool.tile(list(shape), BF16, tag=tag or dram_t.name,
                              **({"bufs": bufs} if bufs else {}))
                src = dram_t.ap()[l]
                if cols is not None:
                    src = src[:, cols]
                nc.sync.dma_start(out=t[:], in_=src.rearrange(rearr, p=128))
                return t

            # Cross-pair x exchange for the next layer's LN1+QKV, in two
            # pieces: (1) post-attention x (bf16), gathered during the MLP
            # (fully hidden); (2) only the MLP delta (fp8), gathered in two
            # feature halves right as the w2 m-tiles finish, so just ~28us
            # of the second half is exposed. The fp8 half also carries the
            # next layer's LN1 row stats (computed from own f32 x) as a
            # bf16-bitcast tail, so the receiver skips the stats pass.
            XQT = 128 * CH                     # elems per feature tile
            FTA = 4 * XQT                      # fp8 delta elems, first AG
            FTB = 4 * XQT                      # fp8 delta elems, second AG
            FST = 4 * CH                       # stats tail (2xCH bf16 bytes)

            def ffd_copy(gin, t, po):
                pof = evac.tile([128, CH], F8, tag="po", bufs=2)
                if t >= 6:
                    nc.vector.tensor_copy(out=pof[:], in_=po[:])
                else:
                    nc.scalar.activation(out=pof[:], in_=po[:],
                                         func=AF.Identity, scale=1.0)
                o = (t % 4) * XQT
                nc.sync.dma_start(
                    out=gin[o:o + XQT].rearrange("(p q) -> p q", p=128),
                    in_=pof[:])

            def gather_go(gin, lnext, half):
                n = (FTB + FST) if half == 1 else FTA
                gout = dram.tile([2, n], F8, tag=f"agxo{half}",
                                 name=f"agxo{lnext}_{half}")
                nc.gpsimd.collective_compute(
                    "AllGather", BYPASS, replica_groups=REPLICA_GROUPS,
                    ins=[gin.opt()], outs=[gout.opt()])
                return gout

            # ---- main body ----------------------------------------------
            agx_prev = None
            for l in range(L):
                # LN1 over both token chunks from the gathered (bf16) x;
                # identical h on both pair members, no h collective.
                hT = big.tile([128, NT, S], BF16, tag="big", name=f"hT{l}")
                for c in range(NCH):
                    xp = acts.tile([128, NT, CH], BF16, tag="h2", bufs=2,
                                   name=f"xp{l}_{c}")

                    def wr_hc(t, tmp, gs, bs, c=c, hT=hT):
                        nc.scalar.activation(
                            out=hT[:, t, c * CH:(c + 1) * CH], in_=tmp[:],
                            func=AF.Identity, scale=gs, bias=bs)

                    if l == 0:
                        nc.sync.dma_start(out=xp[:], in_=x0f_d.ap()[c])
                        st_r = rows.tile([1, CH], BF16, tag="strowr",
                                         bufs=1, name=f"strowr{l}_{c}")
                        st_m = rows.tile([1, CH], BF16, tag="strowm",
                                         bufs=1, name=f"strowm{l}_{c}")
                        nc.sync.dma_start(out=st_r[:],
                                          in_=st0_d.ap()[c, 0:1, :])
                        nc.sync.dma_start(out=st_m[:],
                                          in_=st0_d.ap()[c, 1:2, :])
                        ln_norm((st_r, st_m), g1T, be1T, l, wr_hc, src=xp)
                    else:
                        gxp, ga, gb = agx_prev
                        nc.gpsimd.dma_start(
                            out=xp[:],
                            in_=gxp[c].rearrange("(t p q) -> p t q",
                                                 t=NT, p=128))
                        st_r = rows.tile([1, CH], BF16, tag="strowr",
                                         bufs=1, name=f"strowr{l}_{c}")
                        st_m = rows.tile([1, CH], BF16, tag="strowm",
                                         bufs=1, name=f"strowm{l}_{c}")
                        nc.gpsimd.dma_start(
                            out=st_r[:],
                            in_=gb[c, FTB:FTB + 2 * CH].bitcast(BF16)
                                .rearrange("(r q) -> r q", r=1))
                        nc.gpsimd.dma_start(
                            out=st_m[:],
                            in_=gb[c, FTB + 2 * CH:FTB + FST].bitcast(BF16)
                                .rearrange("(r q) -> r q", r=1))
                        for t in range(NT):
                            g = ga if t < 4 else gb
                            o = (t % 4) * XQT
                            fft = evac.tile([128, CH], F8, tag="po", bufs=2)
                            nc.gpsimd.dma_start(
                                out=fft[:],
                                in_=g[c, o:o + XQT].rearrange("(p q) -> p q",
                                                              p=128))
                            nc.vector.tensor_tensor(
                                out=xp[:, t, :], in0=xp[:, t, :],
                                in1=fft[:], op=ADD)
                        ln_norm((st_r, st_m), g1T, be1T, l, wr_hc, src=xp)

                # per-layer weight tiles (single DMAs, 1KB contiguous rows)
                wqL = load_w(wq_d, l, [128, NT, SH])
                wkL = load_w(wk_d, l, [128, NT, SH])
                wvL = load_w(wv_d, l, [128, NT, SH])
                woL = load_w(wo_d, l, [128, NQ, D])

                # K projection -> feature-major [128, NQ, S] bf16
                # (chunk-major so chunk 0 attention can start early)
                KT = acts.tile([128, NQ, S], BF16, tag="kt")
                for c in range(NCH):
                    for m in range(NQ):
                        cs = slice(c * CH, (c + 1) * CH)
                        ps = pp.tile([128, CH], F32, tag="pp")
                        for k in range(NT):
                            nc.tensor.matmul(
                                ps[:], wkL[:, k, m * 128:(m + 1) * 128],
                                hT[:, k, cs],
                                start=(k == 0), stop=(k == NT - 1))
                        nc.scalar.activation(
                            out=KT[:, m, cs], in_=ps[:], func=AF.Identity,
                            bias=bkT[:, l, m : m + 1], scale=1.0)

                # V projection -> token-major stripes (64 cols + ones col)
                def v_proj(mt):
                    ps = pp.tile([128, SH], F32, tag="pp")
                    for k in range(NT):
                        nc.tensor.matmul(
                            ps[:], hT[:, k, mt * 128:(mt + 1) * 128],
                            wvL[:, k, :],
                            start=(k == 0), stop=(k == NT - 1))
                    nc.vector.tensor_copy(
                        out=Vt[:, mt, :].rearrange("p (h c) -> p h c", h=HL)[:, :, 0:64],
                        in_=ps[:].rearrange("p (h c) -> p h c", h=HL))

                for mt in range(4):
                    v_proj(mt)

                # attention -> attnT [128, NQ, S] bf16, then wo partials
                attnT = acts.tile([128, NQ, S], BF16, tag="at")
                rs_in = dram.tile([NCH, 128, NT, CH], BF16, tag="rsin")
                rs_out = dram.tile([128, NT, CH], BF16, tag="rsout")
                for c in range(NCH):
                    cs = slice(c * CH, (c + 1) * CH)
                    nk = 4 * c + 4
                    QTc = acts.tile([128, NQ, CH], BF16, tag="qt", bufs=1)
                    for m in range(NQ):
                        ps = pp.tile([128, CH], F32, tag="pp")
                        for k in range(NT):
                            nc.tensor.matmul(
                                ps[:], wqL[:, k, m * 128:(m + 1) * 128],
                                hT[:, k, cs],
                                start=(k == 0), stop=(k == NT - 1))
                        nc.vector.tensor_scalar_add(
                            out=QTc[:, m, :], in0=ps[:],
                            scalar1=bqT[:, l, m : m + 1])
                    if c == 0:
                        for mt in range(4, NT):
                            v_proj(mt)
                    for h in range(HL):
                        base = 64 * (h % 2)
                        hp = h // 2
                        wts = []
                        for j in range(nk):
                            pl = pp.tile([128, CH], F32, tag="pp")
                            nc.tensor.matmul(
                                pl[:],
                                KT[base:base + 64, hp, j * 128:(j + 1) * 128],
                                QTc[base:base + 64, hp, :],
                                start=True, stop=True)
                            wt = wtp.tile([128, CH], BF16, tag="wt", bufs=8)
                            nc.scalar.activation(out=wt[:], in_=pl[:],
                                                 func=AF.Exp, scale=0.125)
                            r = j - 4 * c
                            if r >= 0:
                                nc.vector.tensor_tensor(
                                    out=wt[:], in0=wt[:],
                                    in1=masks[:, r, :], op=MULT)
                            wts.append(wt)
                        pa = pav.tile([65, CH], F32, tag="pav")
                        for j in range(nk):
                            nc.tensor.matmul(
                                pa[:], Vt[:, j, 65 * h : 65 * h + 65],
                                wts[j][:],
                                start=(j == 0), stop=(j == nk - 1))
                        rec = rows.tile([1, CH], F32R, tag="rb")
                        nc.vector.reciprocal(out=rec[:], in_=pa[64:65, :])
                        # broadcast 1/denominator across partitions via a
                        # K=1 ones-matmul into the shared bc PSUM bank, then
                        # to SBUF (DVE cannot read two PSUM operands)
                        pbc = prow.tile([64, CH], F32, tag="bc", bufs=1)
                        nc.tensor.matmul(pbc[:], ones_bc[:, 0:64],
                                         rec[:], start=True, stop=True)
                        recb = wtp.tile([64, CH], BF16, tag="recb", bufs=2)
                        nc.vector.tensor_copy(out=recb[:], in_=pbc[:])
                        nc.vector.tensor_tensor(
                            out=attnT[base:base + 64, hp, cs],
                            in0=pa[0:64, :], in1=recb[:], op=MULT)
                    for t in range(NQ):
                        nc.vector.tensor_scalar_add(
                            out=attnT[:, t, cs], in0=attnT[:, t, cs],
                            scalar1=bvT[:, l, t : t + 1])
                    # wo partials for this chunk -> ReduceScatter segment c
                    for m in range(NT):
                        ps = pp.tile([128, CH], F32, tag="pp")
                        for k in range(NQ):
                            nc.tensor.matmul(
                                ps[:], woL[:, k, m * 128:(m + 1) * 128],
                                attnT[:, k, cs],
                                start=(k == 0), stop=(k == NQ - 1))
                        po = evac.tile([128, CH], BF16, tag="po", bufs=2)
                        if c == 0 or m >= 6:
                            nc.scalar.activation(
                                out=po[:], in_=ps[:], func=AF.Identity,
                                bias=boT[:, l, m : m + 1], scale=1.0)
                        else:
                            nc.vector.tensor_scalar_add(
                                out=po[:], in0=ps[:],
                                scalar1=boT[:, l, m : m + 1])
                        nc.sync.dma_start(out=rs_in[c, :, m, :], in_=po[:])
                nc.gpsimd.collective_compute(
                    "ReduceScatter", ADD, replica_groups=REPLICA_GROUPS,
                    ins=[rs_in.opt()], outs=[rs_out.opt()])

                # residual(x) += RS result, then LN2 -> h2 (bf16).
                # Post-attention x is copied out per tile and AllGathered
                # during the MLP (fully hidden).
                rsS = acts.tile([128, NT, CH], BF16, tag="h2", bufs=2,
                                name=f"rsS{l}")
                nc.sync.dma_start(out=rsS[:, 0:4, :], in_=rs_out[:, 0:4, :])
                nc.sync.dma_start(out=rsS[:, 4:8, :], in_=rs_out[:, 4:8, :])
                for t in range(NT):
                    nc.vector.tensor_tensor(
                        out=xT[:, t, :], in0=xT[:, t, :],
                        in1=rsS[:, t, :], op=ADD)
                h2 = acts.tile([128, NT, CH], BF16, tag="h2", bufs=2)

                def wr_h2(t, tmp, gs, bs, h2=h2):
                    nc.vector.tensor_scalar(
                        out=h2[:, t, :], in0=tmp[:],
                        scalar1=gs, scalar2=bs, op0=MULT, op1=ADD)

                layernorm(g2T, be2T, l, wr_h2)

                # post-attention x copy-out + hidden AllGather (during MLP)
                if l + 1 < L:
                    gxp_in = dram.tile([NT * XQT], BF16, tag="gxpi",
                                       name=f"gxpi{l + 1}")
                    for t in range(NT):
                        xc = evac.tile([128, CH], BF16, tag="po", bufs=2)
                        nc.scalar.activation(out=xc[:], in_=xT[:, t, :],
                                             func=AF.Identity, scale=1.0)
                        nc.sync.dma_start(
                            out=gxp_in[t * XQT:(t + 1) * XQT]
                                .rearrange("(p q) -> p q", p=128),
                            in_=xc[:])
                    gxp_out = dram.tile([2, NT * XQT], BF16, tag="gxpo",
                                        name=f"gxpo{l + 1}")
                    nc.gpsimd.collective_compute(
                        "AllGather", BYPASS, replica_groups=REPLICA_GROUPS,
                        ins=[gxp_in.opt()], outs=[gxp_out.opt()])

                # MLP: full 4096-wide FF over own 512 tokens, no collective
                gT = big.tile([128, NM, CH], BF16, tag="big", name=f"gT{l}")
                for mm in range(16):
                    w1t = w1s.tile([128, NT, 256], BF16, tag="w1")
                    nc.sync.dma_start(
                        out=w1t[:],
                        in_=w1_d.ap()[l][:, mm * 256:(mm + 1) * 256]
                            .rearrange("(t p) m -> p t m", p=128))
                    for ms in range(2):
                        m = mm * 2 + ms
                        ps = pp.tile([128, CH], F32, tag="pp")
                        for k in range(NT):
                            nc.tensor.matmul(
                                ps[:], w1t[:, k, ms * 128:(ms + 1) * 128],
                                h2[:, k, :],
                                start=(k == 0), stop=(k == NT - 1))
                        nc.scalar.activation(
                            out=gT[:, m, :], in_=ps[:], func=AF.Gelu,
                            bias=b1T[:, l, m : m + 1], scale=1.0)
                agx_a = None
                if l + 1 < L:
                    gin_a = dram.tile([FTA], F8, tag="agxi0",
                                      name=f"agxi{l + 1}_0")
                    gin_b = dram.tile([FTB + FST], F8, tag="agxi1",
                                      name=f"agxi{l + 1}_1")
                    psxN = prow.tile([1, CH], F32, tag="psx")
                    psx2N = prow.tile([1, CH], F32, tag="psx2")
                for m2 in range(4):
                    w2t = w2s.tile([128, NM, 256], BF16, tag="w2")
                    nc.sync.dma_start(
                        out=w2t[:],
                        in_=w2_d.ap()[l][:, m2 * 256:(m2 + 1) * 256]
                            .rearrange("(t p) m -> p t m", p=128))
                    for ms in range(2):
                        m = m2 * 2 + ms
                        ps = pp.tile([128, CH], F32, tag="pp")
                        for k in range(NM):
                            nc.tensor.matmul(
                                ps[:], w2t[:, k, ms * 128:(ms + 1) * 128],
                                gT[:, k, :],
                                start=(k == 0), stop=(k == NM - 1))
                        po = evac.tile([128, CH], BF16, tag="po", bufs=2)
                        nc.scalar.activation(
                            out=po[:], in_=ps[:], func=AF.Identity,
                            bias=b2T[:, l, m : m + 1], scale=1.0)
                        nc.vector.tensor_tensor(
                            out=xT[:, m, :], in0=xT[:, m, :],
                            in1=po[:], op=ADD)
                        if l + 1 < L:
                            ffd_copy(gin_a if m < 4 else gin_b, m, po)
                            nc.tensor.matmul(psxN[:], ones_k, xT[:, m, :],
                                             start=(m == 0),
                                             stop=(m == NT - 1))
                            sq = evac.tile([128, CH], BF16, tag="lntmp",
                                           bufs=3)
                            nc.vector.tensor_tensor(out=sq[:],
                                                    in0=xT[:, m, :],
                                                    in1=xT[:, m, :],
                                                    op=MULT)
                            nc.tensor.matmul(psx2N[:], ones_kb, sq[:],
                                             start=(m == 0),
                                             stop=(m == NT - 1))
                            if m == 3:
                                agx_a = gather_go(gin_a, l + 1, 0)
                if l + 1 < L:
                    stb_r = rows.tile([1, CH], BF16, tag="stbr", bufs=1,
                                      name=f"stbr{l + 1}")
                    stb_m = rows.tile([1, CH], BF16, tag="stbm", bufs=1,
                                      name=f"stbm{l + 1}")
                    ln_rowstats(psxN, psx2N, (stb_r, stb_m))
                    nc.sync.dma_start(
                        out=gin_b[FTB:FTB + 2 * CH].bitcast(BF16)
                            .rearrange("(r q) -> r q", r=1),
                        in_=stb_r[:])
                    nc.sync.dma_start(
                        out=gin_b[FTB + 2 * CH:FTB + FST].bitcast(BF16)
                            .rearrange("(r q) -> r q", r=1),
                        in_=stb_m[:])
                    agx_prev = (gxp_out, agx_a,
                                gather_go(gin_b, l + 1, 1))

            # final LN -> output (own token half)
            def wr_out(t, tmp, gs, bs):
                ot = evac.tile([128, CH], F32, tag="ot", bufs=1)
                nc.vector.tensor_scalar(out=ot[:], in0=tmp[:],
                                        scalar1=gs, scalar2=bs,
                                        op0=MULT, op1=ADD)
                nc.sync.dma_start(out=out_v[:, t, :], in_=ot[:])

            layernorm(gfT, befT, None, wr_out)

    return nc


# ---------------------------------------------------------------------------
# host side
# ---------------------------------------------------------------------------

def _sinusoidal_pe(s, d):
    pos = np.arange(s, dtype=np.float32)[:, None]
    div = np.exp(np.arange(0, d, 2, dtype=np.float32)
                 * np.float32(-np.log(10000.0) / d)).astype(np.float32)
    pe = np.zeros((s, d), dtype=np.float32)
    pe[:, 0::2] = np.sin(pos * div)
    pe[:, 1::2] = np.cos(pos * div)
    return pe


def _pp128(v):
    """[L?, n*128] -> [L?, 128, n] with feature = 128*m + p."""
    v = np.asarray(v, dtype=np.float32)
    if v.ndim == 1:
        return np.ascontiguousarray(v.reshape(-1, 128).T)
    lq, n = v.shape
    return np.ascontiguousarray(v.reshape(lq, n // 128, 128).transpose(0, 2, 1))


_NC_CACHE = {}


def _get_nc(repeat=1):
    if repeat not in _NC_CACHE:
        _NC_CACHE[repeat] = build_nc()
    return _NC_CACHE[repeat]


def make_in_maps(input_ids, tok_emb, wq, bq, wk, bk, wv, bv, wo, bo,
                 ln1_g, ln1_b, ln2_g, ln2_b, w1, b1, w2, b2, lnf_g, lnf_b):
    input_ids = np.asarray(input_ids)
    pe = _sinusoidal_pe(S, D)
    masks = np.zeros((128, 4, CH), dtype=np.float32)
    ar = np.arange(CH)
    for r in range(4):
        for p in range(128):
            masks[p, r, :] = (ar >= 128 * r + p).astype(np.float32)
    cones = np.ones((128, 128), dtype=np.float32)
    bf = ml_dtypes.bfloat16

    in_maps = []
    for core in range(N_CORES):
        b = core // 2
        j = core % 2
        qs = slice(j * SH, (j + 1) * SH)
        ts = slice(j * CH, (j + 1) * CH)
        x0 = (tok_emb[input_ids[b]] + pe).astype(np.float32)   # [S, D]
        x0f = np.ascontiguousarray(
            x0.T.reshape(NT, 128, NCH, CH).transpose(2, 1, 0, 3)).astype(bf)
        m0 = x0.mean(axis=1)
        r0 = 1.0 / np.sqrt(x0.var(axis=1) + 1e-5)
        st0 = np.stack([r0, m0 * r0]).astype(np.float32)       # [2, S]
        st0 = np.ascontiguousarray(
            st0.reshape(2, NCH, CH).transpose(1, 0, 2)).astype(bf)
        m = {
            "x0t": np.ascontiguousarray(x0.T[:, ts]),
            "x0f": x0f,
            "st0": st0,
            "wq": np.ascontiguousarray(wq[:, :, qs]).astype(bf),
            "wk": np.ascontiguousarray(wk[:, :, qs]).astype(bf),
            "wv": np.ascontiguousarray(wv[:, :, qs]).astype(bf),
            "wo": np.ascontiguousarray(wo[:, qs, :]).astype(bf),
            "w1": np.ascontiguousarray(w1).astype(bf),
            "w2": np.ascontiguousarray(w2).astype(bf),
            "bq": _pp128(bq[:, qs]),
            "bk": _pp128(bk[:, qs]),
            "bv": _pp128(bv[:, qs]),
            "bo2": _pp128(bo * 0.5),
            "b1": _pp128(b1),
            "b2f": _pp128(b2),
            "g1": _pp128(ln1_g),
            "be1": _pp128(ln1_b),
            "g2": _pp128(ln2_g),
            "be2": _pp128(ln2_b),
            "gf": _pp128(lnf_g),
            "bef": _pp128(lnf_b),
            "masks": masks,
            "cones": cones,
        }
        in_maps.append(m)
    return in_maps


def kernel(input_ids, attention_mask, tok_emb, ln1_g, ln1_b, wq, bq, wk, bk,
           wv, bv, wo, bo, ln2_g, ln2_b, w1, b1, w2, b2, lnf_g, lnf_b,
           _repeat=1):
    args = [np.asarray(a, dtype=np.float32) for a in
            (tok_emb, wq, bq, wk, bk, wv, bv, wo, bo,
             ln1_g, ln1_b, ln2_g, ln2_b, w1, b1, w2, b2, lnf_g, lnf_b)]
    (tok_emb, wq, bq, wk, bk, wv, bv, wo, bo,
     ln1_g, ln1_b, ln2_g, ln2_b, w1, b1, w2, b2, lnf_g, lnf_b) = args
    in_maps = make_in_maps(input_ids, tok_emb, wq, bq, wk, bk, wv, bv, wo, bo,
                           ln1_g, ln1_b, ln2_g, ln2_b, w1, b1, w2, b2,
                           lnf_g, lnf_b)
    nc = _get_nc(_repeat)
    res = run_bass_kernel_spmd(nc, in_maps, list(range(N_CORES)))
    out = np.empty((B, S, D), dtype=np.float32)
    for b in range(B):
        for j in range(2):
            out[b, j * CH:(j + 1) * CH] = res.results[2 * b + j]["outt"].T
    return out
